# revision 1
# baseline (speedup 1.0000x reference)
"""EGConv layer (gnn_message_passing) on 8 Trainium2 NeuronCores.

Self-contained: kernel(**inputs) -> np.ndarray [50000, 256] float32.

Strategy: graph-aligned 1D node partition over 8 cores (GraphNorm fully
core-local), per-core degree-sorted node permutation, dst-sorted edge
streams; messages = bf16 bases rows fetched by dma_gather from a
two-way-split STRIPED bases table (int16 index range, 2KB-contiguous
writes); sum/sym aggregation via block-level one-hot matmuls on the
tensor engine (plain + symw-weighted one-hots, gather tile as rhs);
max via slot-layout gather + halve + strided max-reduce; per-node
einsum in bf16 on the vector engine with a tree reduce; GraphNorm via
per-graph one-hot matmuls, h kept resident in SBUF. The SPMD program
is identical across cores; all per-core variation is in the data.
"""
import sys
for _p in ("/opt/trn_rl_repo", "/root/.axon_site/_ro/trn_rl_repo"):
    if _p not in sys.path:
        sys.path.insert(0, _p)

import os
import numpy as np
import ml_dtypes
from contextlib import ExitStack

import concourse.bass as bass
import concourse.mybir as mybir
import concourse.tile as tile
from concourse import bacc, bass_utils
from concourse.masks import make_identity

BFNP = ml_dtypes.bfloat16

# ======================= host-side graph preprocessing =======================

N, E, D = 50000, 800000, 256
H, B, A = 8, 4, 3
F = D // H          # 32
BF = B * F          # 128
G = 64
EPS = 1e-5
NCORES = 8
P = 128
NEG = -1e30
SPLIT = 32640     # bases table row split (int16 index limit); 255*128
PAD_ROW = ((N + P - 1) // P) * P          # 50048
SP = SPLIT                                 # 32640
NC0 = SP // P                              # 255 striped cols, table 0
NC1 = (PAD_ROW - SP) // P                  # 136 striped cols, table 1


def _srow0(src):
    """original row id -> striped row id, table 0."""
    return (src % P) * NC0 + src // P


def _srow1(src):
    s = src - SP
    return (s % P) * NC1 + s // P


def build(edge_index: np.ndarray, batch: np.ndarray):
    """edge_index [2,E] int32, batch [N] int32 sorted. Returns layout dict."""
    src_all = np.concatenate([edge_index[0], np.arange(N, dtype=np.int64)])
    dst_all = np.concatenate([edge_index[1], np.arange(N, dtype=np.int64)])

    deg = np.bincount(dst_all, minlength=N).astype(np.float64)
    dinv = np.where(deg > 0, 1.0 / np.sqrt(deg), 0.0).astype(np.float32)
    symw_all = (dinv[src_all] * dinv[dst_all]).astype(np.float32)

    # graph-aligned 8-way shard
    gcnt = np.bincount(batch, minlength=G)
    gend = np.cumsum(gcnt)            # node index where graph g ends
    cuts = [0]
    for c in range(1, NCORES):
        target = round(N * c / NCORES)
        gi = np.argmin(np.abs(gend - target))
        cuts.append(int(gend[gi]))
    cuts.append(N)
    cuts = sorted(set(cuts))
    assert len(cuts) == NCORES + 1, cuts

    cores = []
    for c in range(NCORES):
        n0, n1 = cuts[c], cuts[c + 1]
        nloc = n1 - n0
        local_deg = deg[n0:n1]
        # secondary key: range-0 in-degree, to tighten per-range slot
        # rectangles (removes binomial-thinning variance within blocks)
        ldeg0 = np.bincount(dst_all[(dst_all >= n0) & (dst_all < n1)
                                    & (src_all < SPLIT)] - n0,
                            minlength=n1 - n0).astype(np.float64)
        perm = np.lexsort((-ldeg0, -local_deg)).astype(np.int64)  # desc
        gperm = perm + n0                      # new local id -> global id
        inv = np.empty(nloc, dtype=np.int64)
        inv[perm] = np.arange(nloc)            # global-local -> new local id

        emask = (dst_all >= n0) & (dst_all < n1)
        esrc = src_all[emask]
        edstl = inv[dst_all[emask] - n0]       # new local dst id
        esym = symw_all[emask]
        order = np.argsort(edstl, kind="stable")
        esrc, edstl, esym = esrc[order], edstl[order], esym[order]

        cores.append(dict(n0=n0, n1=n1, nloc=nloc, gperm=gperm,
                          esrc=esrc, edstl=edstl, esym=esym,
                          ldeg=deg[gperm].astype(np.int64)))

    nblk = max((c["nloc"] + P - 1) // P for c in cores)

    # per-(core, block, range) tile/slot counts, maxed across cores
    nR = 2
    Tr = np.zeros((nR, nblk), dtype=np.int64)
    Sr = np.zeros((nR, nblk), dtype=np.int64)
    for c in cores:
        blk = c["edstl"] // P
        rng = (c["esrc"] >= SP).astype(np.int64)
        for r in range(nR):
            cnt = np.bincount(blk[rng == r], minlength=nblk)
            Tr[r] = np.maximum(Tr[r], (cnt + P - 1) // P)
            dl = c["edstl"][rng == r]
            nd = np.bincount(dl, minlength=nblk * P).reshape(nblk, P)
            Sr[r] = np.maximum(Sr[r], nd.max(axis=1))
    Tr = np.maximum(Tr, 1)
    Sr = np.maximum(Sr, 1)

    PAD0, PAD1 = SP, PAD_ROW - SP   # pad-row striped index in T0 / T1

    sumTT = int((Tr[0] + Tr[1]).sum())
    for c in cores:
        dstl_t = np.full((P, sumTT), -1.0, dtype=BFNP)
        symw_t = np.zeros((P, sumTT), dtype=np.float32)
        gid_t = np.full((P, nblk), -1.0, dtype=np.float32)
        # per-range gather index streams (edge tiles then slots per block)
        flat_r = [[], []]
        blk = c["edstl"] // P
        rng = (c["esrc"] >= SP).astype(np.int64)
        # self-loop row of each local node (for slot pads when the node
        # has edges in the range): global id -> striped row in own range
        tcol = 0
        for b in range(nblk):
            for r in range(nR):
                m = (blk == b) & (rng == r)
                src = c["esrc"][m]
                srow = _srow1(src) if r else _srow0(src)
                dl = c["edstl"][m] - b * P
                sw = c["esym"][m]
                k = len(src)
                T, S = int(Tr[r][b]), int(Sr[r][b])
                pad = PAD1 if r else PAD0
                ef = np.full(P * T, pad, dtype=np.int64)
                ef[:k] = srow
                flat_r[r].append(ef)
                cols = tcol + np.arange(k) // P
                rows = np.arange(k) % P
                dstl_t[rows, cols] = dl.astype(np.float32)
                symw_t[rows, cols] = sw
                tcol += T
                sf = np.full(P * S, pad, dtype=np.int64)
                if k:
                    marks = np.flatnonzero(np.diff(dl, prepend=-1))
                    slot = np.arange(k) - np.repeat(marks, np.diff(
                        np.append(marks, k)))
                    sf[slot * P + dl] = srow
                    # pad slots of nodes that HAVE >=1 edge in this range:
                    # duplicate the node's first edge (max unchanged, and
                    # avoids a pad-row fetch)
                    first = np.full(P, -1, dtype=np.int64)
                    first[dl[marks]] = srow[marks]
                    degr = np.zeros(P, dtype=np.int64)
                    dcnt = np.diff(np.append(marks, k))
                    degr[dl[marks]] = dcnt
                    for s in range(S):
                        lane = np.flatnonzero((degr > 0) & (degr <= s))
                        sf[s * P + lane] = first[lane]
                flat_r[r].append(sf)
        i16 = []
        for r in range(nR):
            fl = np.concatenate(flat_r[r])
            L = len(fl)
            w = np.zeros((16, L // 16), dtype=np.int16)
            w[np.arange(L) % 16, np.arange(L) // 16] = fl
            i16.append(np.tile(w, (8, 1)))

        ngid = np.full(nblk * P, -1.0, dtype=np.float32)
        gl0 = batch[c["n0"]]
        ngid[:c["nloc"]] = (batch[c["gperm"]] - gl0).astype(np.float32)
        gid_t[:] = ngid.reshape(nblk, P).T

        icnt = np.ones((G, 1), dtype=np.float32)
        glo = np.bincount((batch[c["n0"]:c["n1"]] - gl0), minlength=G)
        icnt[glo > 0, 0] = (1.0 / glo[glo > 0]).astype(np.float32)

        c.update(dstl_t=dstl_t, symw_t=symw_t, i16_0=i16[0], i16_1=i16[1],
                 gid_t=gid_t, invcnt=icnt)

    return dict(cores=cores, nblk=nblk, Tr=Tr, Sr=Sr, cuts=cuts)


def unshard(layout, per_core_out):
    full = np.zeros((N, D), dtype=np.float32)
    for c, out in zip(layout["cores"], per_core_out):
        full[c["gperm"]] = out[:c["nloc"]]
    return full

# ============ input-map construction ============


def to_bf16(x):
    return np.asarray(x, np.float32).astype(BFNP)


def make_inputs(inputs, lay):
    """inputs: dict of full np arrays. lay: build output.
    Returns (meta, in_maps list of 8 dicts)."""
    nv = PAD_ROW
    nblk = lay["nblk"]
    npad = nblk * P

    node = np.asarray(inputs["node"], np.float32)
    nodeT = np.zeros((D, nv), BFNP)
    nodeT[:, :N] = to_bf16(node).T
    wb = to_bf16(inputs["W_bases"])                       # [D, BF]
    wc = to_bf16(inputs["W_comb"])                        # [D, HBA]
    bcomb = np.tile(np.asarray(inputs["b_comb"], np.float32)[None, :], (P, 1))
    cbias = np.tile(np.asarray(inputs["conv_bias"], np.float32)[None, :], (P, 1))
    alphar = np.tile(np.asarray(inputs["gn_mean_scale"], np.float32)[None, :], (G, 1))
    gammar = np.tile(np.asarray(inputs["gn_weight"], np.float32)[None, :], (G, 1))
    br = np.tile(np.asarray(inputs["gn_bias"], np.float32)[None, :], (G, 1))
    padrow = np.full((1, BF), NEG, BFNP)

    meta = dict(nv=nv, nblk=nblk,
                Tr0=[int(x) for x in lay["Tr"][0]],
                Tr1=[int(x) for x in lay["Tr"][1]],
                Sr0=[int(x) for x in lay["Sr"][0]],
                Sr1=[int(x) for x in lay["Sr"][1]])

    in_maps = []
    for c in lay["cores"]:
        ntl = np.zeros((D, npad), BFNP)
        ntl[:, :c["nloc"]] = to_bf16(node[c["gperm"]]).T
        in_maps.append(dict(
            nodeT=nodeT, nodeTloc=ntl, wb=wb, wc=wc, bcomb=bcomb,
            dstl=c["dstl_t"], symw=c["symw_t"],
            i16_0=c["i16_0"], i16_1=c["i16_1"], gid=c["gid_t"],
            invc=np.pad(c["invcnt"], ((0, G - c["invcnt"].shape[0]), (0, 0)),
                        constant_values=1.0),
            alphar=alphar, gammar=gammar, br=br, cbias=cbias,
            padrow=padrow))
    return meta, in_maps

# ============ device program ============

FP32 = mybir.dt.float32
BF16 = mybir.dt.bfloat16
I32 = mybir.dt.int32
I16 = mybir.dt.int16
AX = mybir.AxisListType
OP = mybir.AluOpType
ACTF = mybir.ActivationFunctionType
HBA = H * B * A   # 96
K = B * A         # 12


def build_program(nc, meta):
    nv = meta["nv"]
    nblk = meta["nblk"]
    Tr0, Tr1 = list(meta["Tr0"]), list(meta["Tr1"])
    Sr0, Sr1 = list(meta["Sr0"]), list(meta["Sr1"])
    sumT = sum(Tr0) + sum(Tr1)
    L0 = sum(P * (t + s) for t, s in zip(Tr0, Sr0)) // 16
    L1 = sum(P * (t + s) for t, s in zip(Tr1, Sr1)) // 16
    npad = nblk * P
    TTmax = max(t0 + t1 for t0, t1 in zip(Tr0, Tr1))
    ntt = nv // P                     # node tiles for bases stage (391)

    # ---- external tensors -------------------------------------------------
    nodeT = nc.dram_tensor("nodeT", [D, nv], BF16, kind="ExternalInput")
    nodeTloc = nc.dram_tensor("nodeTloc", [D, npad], BF16, kind="ExternalInput")
    wb = nc.dram_tensor("wb", [D, BF], BF16, kind="ExternalInput")
    wc = nc.dram_tensor("wc", [D, HBA], BF16, kind="ExternalInput")
    bcomb = nc.dram_tensor("bcomb", [P, HBA], FP32, kind="ExternalInput")
    dstl = nc.dram_tensor("dstl", [P, sumT], BF16, kind="ExternalInput")
    symw = nc.dram_tensor("symw", [P, sumT], FP32, kind="ExternalInput")
    i16_0 = nc.dram_tensor("i16_0", [P, L0], I16, kind="ExternalInput")
    i16_1 = nc.dram_tensor("i16_1", [P, L1], I16, kind="ExternalInput")
    gid = nc.dram_tensor("gid", [P, nblk], FP32, kind="ExternalInput")
    invc = nc.dram_tensor("invc", [G, 1], FP32, kind="ExternalInput")
    alphar = nc.dram_tensor("alphar", [G, D], FP32, kind="ExternalInput")
    gammar = nc.dram_tensor("gammar", [G, D], FP32, kind="ExternalInput")
    br = nc.dram_tensor("br", [G, D], FP32, kind="ExternalInput")
    cbias = nc.dram_tensor("cbias", [P, D], FP32, kind="ExternalInput")
    padrow = nc.dram_tensor("padrow", [1, BF], BF16, kind="ExternalInput")
    hout = nc.dram_tensor("hout", [npad, D], FP32, kind="ExternalOutput")
    DBG = os.environ.get("K_DBG") == "1"
    if DBG:
        W0d = meta["Tr0"][0] + meta["Sr0"][0]
        W1d = meta["Tr1"][0] + meta["Sr1"][0]
        TTd = meta["Tr0"][0] + meta["Tr1"][0]
        d_tab0 = nc.dram_tensor("d_tab0", [SP + P, BF], BF16, kind="ExternalOutput")
        d_gath = nc.dram_tensor("d_gath", [P, (W0d + W1d) * BF], BF16, kind="ExternalOutput")
        d_oh = nc.dram_tensor("d_oh", [P, P * TTd], BF16, kind="ExternalOutput")
        d_ohw = nc.dram_tensor("d_ohw", [P, P * TTd], BF16, kind="ExternalOutput")
        d_ps = nc.dram_tensor("d_ps", [P, 2 * BF], FP32, kind="ExternalOutput")
        d_aggT = nc.dram_tensor("d_aggT", [P, F * K], BF16, kind="ExternalOutput")
        d_comb = nc.dram_tensor("d_comb", [P, nblk * HBA], BF16, kind="ExternalOutput")
        d_hb = nc.dram_tensor("d_hb", [P, nblk * D], BF16, kind="ExternalOutput")

    with ExitStack() as ctx:
        tc = ctx.enter_context(tile.TileContext(nc))
        dram = ctx.enter_context(tc.tile_pool(name="dram", bufs=1, space="DRAM"))
        res = ctx.enter_context(tc.tile_pool(name="res", bufs=1))
        pa = ctx.enter_context(tc.tile_pool(name="pa", bufs=3))
        pgath = ctx.enter_context(tc.tile_pool(name="pgath", bufs=2))
        pidx = ctx.enter_context(tc.tile_pool(name="pidx", bufs=2))
        poh = ctx.enter_context(tc.tile_pool(name="poh", bufs=2))
        ptmp = ctx.enter_context(tc.tile_pool(name="ptmp", bufs=2))
        psm = ctx.enter_context(tc.tile_pool(name="psm", bufs=2))
        pd = ctx.enter_context(tc.tile_pool(name="pd", bufs=1))

        bases0 = dram.tile([SP + P, BF], BF16)        # + pad row at SP
        bases1 = dram.tile([nv - SP + P, BF], BF16)   # + pad row at nv-SP

        # ---- constants / resident tiles ----------------------------------
        wb_sb = res.tile([P, 2, BF], BF16)
        nc.sync.dma_start(wb_sb[:], wb.ap().rearrange("(a p) f -> p a f", p=P))
        wc_sb = res.tile([P, 2, HBA], BF16)
        nc.sync.dma_start(wc_sb[:], wc.ap().rearrange("(a p) f -> p a f", p=P))
        bcomb_sb = res.tile([P, HBA], FP32)
        nc.sync.dma_start(bcomb_sb[:], bcomb.ap())
        dstl_sb = res.tile([P, sumT], BF16)
        nc.sync.dma_start(dstl_sb[:], dstl.ap())
        symw_sb = res.tile([P, sumT], FP32)
        nc.sync.dma_start(symw_sb[:], symw.ap())
        gid_sb = res.tile([P, nblk], FP32)
        nc.sync.dma_start(gid_sb[:], gid.ap())
        invc_sb = res.tile([G, 1], FP32)
        nc.sync.dma_start(invc_sb[:], invc.ap())
        alphar_sb = res.tile([G, D], FP32)
        nc.sync.dma_start(alphar_sb[:], alphar.ap())
        gammar_sb = res.tile([G, D], FP32)
        nc.sync.dma_start(gammar_sb[:], gammar.ap())
        br_sb = res.tile([G, D], FP32)
        nc.sync.dma_start(br_sb[:], br.ap())
        cbias_sb = res.tile([P, D], FP32)
        nc.sync.dma_start(cbias_sb[:], cbias.ap())
        # pad rows of the bases tables (written before any gather reads them)
        nc.sync.dma_start(bases0[SP:SP + 1, :], padrow.ap())
        nc.sync.dma_start(bases1[nv - SP:nv - SP + 1, :], padrow.ap())

        ident = res.tile([P, P], FP32)
        make_identity(nc, ident[:])
        ident_bf = res.tile([P, P], BF16)
        nc.vector.tensor_copy(ident_bf[:], ident[:])
        iota_i = res.tile([P, P], I32)
        nc.gpsimd.iota(iota_i[:], pattern=[[1, P]], base=0, channel_multiplier=0)
        iota_f = res.tile([P, P], FP32)
        nc.vector.tensor_copy(iota_f[:], iota_i[:])
        # iota_exp[p, x, t] = x  (bf16, for block-level one-hot builds)
        iota_bf = res.tile([P, P], BF16)
        nc.vector.tensor_copy(iota_bf[:], iota_i[:])
        iota_exp = res.tile([P, P, TTmax], BF16)
        nc.scalar.copy(iota_exp[:],
                       iota_bf[:].unsqueeze(2).broadcast_to([P, P, TTmax]))

        comb_sb = res.tile([P, nblk, HBA], BF16)
        goh_all = res.tile([P, nblk, G], BF16)
        hb_all = res.tile([P, nblk, D], BF16)

        # ---- stage A: full bases table (striped layout) -------------------
        # comb matmuls interleaved so PE/DVE fill while stage-A DMA streams.
        pab_cm = tc.tile_pool(name="pab", bufs=4, space="PSUM")
        pab = pab_cm.__enter__()
        pcb_cm = tc.tile_pool(name="pcb", bufs=2, space="PSUM")
        pcb = pcb_cm.__enter__()

        def comb_block(b):
            lt2 = pa.tile([P, 2, P], BF16, tag="ltloc")
            nc.sync.dma_start(lt2[:], nodeTloc.ap().rearrange(
                "(a p) n -> p a n", p=P)[:, :, b * P:(b + 1) * P])
            cps = pcb.tile([P, HBA], FP32, tag="cps")
            nc.tensor.matmul(cps[:], lt2[:, 0, :], wc_sb[:, 0, :],
                             start=True, stop=False)
            nc.tensor.matmul(cps[:], lt2[:, 1, :], wc_sb[:, 1, :],
                             start=False, stop=True)
            nc.vector.tensor_tensor(comb_sb[:, b, :], cps[:], bcomb_sb[:],
                                    op=OP.add)

        CHN = 8
        chunks = [(i0, min(CHN, NC0 - i0), 0) for i0 in range(0, NC0, CHN)]
        chunks += [(i0, min(CHN, NC1 - i0), 1) for i0 in range(0, NC1, CHN)]
        cb_next = 0
        for ci, (i0, cn, tbl) in enumerate(chunks):
            gcol = i0 + (NC0 if tbl else 0)          # global node-tile index
            lt = pa.tile([P, 2, CHN * P], BF16, tag="ntile")
            nc.sync.dma_start(lt[:, :, :cn * P], nodeT.ap().rearrange(
                "(a p) n -> p a n", p=P)[:, :, gcol * P:(gcol + cn) * P])
            ob = pa.tile([P, CHN, BF], BF16, tag="bout")
            for j0 in range(0, cn, 4):
                jn = min(4, cn - j0)
                ps = pab.tile([P, 4, BF], FP32, tag="ab")
                for j in range(jn):
                    nc.tensor.matmul(ps[:, j, :],
                                     lt[:, 0, ((j0 + j) * P):((j0 + j + 1) * P)],
                                     wb_sb[:, 0, :], start=True, stop=False)
                    nc.tensor.matmul(ps[:, j, :],
                                     lt[:, 1, ((j0 + j) * P):((j0 + j + 1) * P)],
                                     wb_sb[:, 1, :], start=False, stop=True)
                if (ci + j0) % 2 == 0:
                    nc.vector.tensor_copy(ob[:, j0:j0 + jn, :], ps[:, :jn, :])
                else:
                    nc.scalar.copy(ob[:, j0:j0 + jn, :], ps[:, :jn, :])
            # striped write: row (src%128)*NC + src//128 -> per-partition
            # contiguous cn*256B runs
            tb_t = bases1 if tbl else bases0
            ncols = NC1 if tbl else NC0
            nc.sync.dma_start(
                tb_t[0:ncols * P, :].rearrange("(p c) f -> p c f", p=P)
                [:, i0:i0 + cn, :], ob[:, :cn, :])
            # interleave ~one comb block per chunk
            while cb_next < nblk and cb_next <= ci:
                comb_block(cb_next)
                cb_next += 1
        while cb_next < nblk:
            comb_block(cb_next)
            cb_next += 1

        pcb_cm.__exit__(None, None, None)
        pab_cm.__exit__(None, None, None)

        # ---- stage C: gather + aggregate + einsum + stats -----------------
        pacc_cm = tc.tile_pool(name="pacc", bufs=1, space="PSUM")
        pacc = pacc_cm.__enter__()
        pagg_cm = tc.tile_pool(name="pagg", bufs=2, space="PSUM")
        pagg = pagg_cm.__enter__()
        gsum_ps = pacc.tile([G, D], FP32)
        gsq_ps = pacc.tile([G, D], FP32)

        CH = 64                       # <=8192 idxs per dma_gather call
        c0 = 0
        c1 = 0
        tb = 0
        for b in range(nblk):
            T0, T1 = Tr0[b], Tr1[b]
            S0, S1 = Sr0[b], Sr1[b]
            W0, W1 = T0 + S0, T1 + S1
            TT = T0 + T1
            gath = pgath.tile([P, W0 + W1, BF], BF16, tag="gath")
            if b < 2:
                nc.gpsimd.memset(gath[:], 0.0)
            # stream the idx slices for this block from DRAM
            ix0 = pidx.tile([P, 8 * W0], I16, tag="ix0")
            nc.sync.dma_start(ix0[:], i16_0.ap()[:, c0:c0 + 8 * W0])
            ix1 = pidx.tile([P, 8 * W1], I16, tag="ix1")
            nc.sync.dma_start(ix1[:], i16_1.ap()[:, c1:c1 + 8 * W1])
            for w0 in range(0, W0, CH):
                w = min(CH, W0 - w0)
                nc.gpsimd.dma_gather(
                    out_ap=gath[:, w0:w0 + w, :], in_ap=bases0[:],
                    idxs_ap=ix0[:, 8 * w0:8 * (w0 + w)],
                    num_idxs=P * w, num_idxs_reg=P * w, elem_size=BF,
                    single_packet=False)
            for w1 in range(0, W1, CH):
                w = min(CH, W1 - w1)
                nc.gpsimd.dma_gather(
                    out_ap=gath[:, W0 + w1:W0 + w1 + w, :], in_ap=bases1[:],
                    idxs_ap=ix1[:, 8 * w1:8 * (w1 + w)],
                    num_idxs=P * w, num_idxs_reg=P * w, elem_size=BF,
                    single_packet=False)
            c0 += 8 * W0
            c1 += 8 * W1

            # block-level one-hot builds: oh[p_edge, x, t]
            oh = poh.tile([P, P, TTmax], BF16, tag="oh")
            nc.vector.tensor_tensor(
                oh[:, :, :TT],
                dstl_sb[:, tb:tb + TT].unsqueeze(1).broadcast_to([P, P, TT]),
                iota_exp[:, :, :TT], op=OP.is_equal)
            ohw = poh.tile([P, P, TTmax], BF16, tag="ohw")
            for t in range(TT):
                nc.scalar.mul(ohw[:, :, t], oh[:, :, t],
                              symw_sb[:, tb + t:tb + t + 1])

            ps_s = pagg.tile([P, 4, BF], FP32, tag="aggsum")
            ps_w = pagg.tile([P, 4, BF], FP32, tag="aggsym")
            for t in range(TT):
                mcol = t if t < T0 else W0 + (t - T0)
                nc.tensor.matmul(ps_s[:, 0, :], oh[:, :, t], gath[:, mcol, :],
                                 start=(t == 0), stop=(t == TT - 1))
                nc.tensor.matmul(ps_w[:, 0, :], ohw[:, :, t], gath[:, mcol, :],
                                 start=(t == 0), stop=(t == TT - 1))

            # max: halve (overlap-safe) then one strided reduce over both
            # ranges into aggT[:, :, 8:12]
            m0, m1 = (S0 + 1) // 2, (S1 + 1) // 2
            hmax = ptmp.tile([P, m0 + m1, BF], BF16, tag="hmax")
            nc.vector.tensor_tensor(hmax[:, :m0, :],
                                    gath[:, T0:T0 + m0, :],
                                    gath[:, T0 + S0 - m0:T0 + S0, :],
                                    op=OP.max)
            nc.vector.tensor_tensor(hmax[:, m0:m0 + m1, :],
                                    gath[:, W0 + T1:W0 + T1 + m1, :],
                                    gath[:, W0 + W1 - m1:W0 + W1, :],
                                    op=OP.max)
            if DBG and b == 0:
                nc.sync.dma_start(d_gath.ap(), gath[:].rearrange("p w f -> p (w f)"))
                nc.sync.dma_start(d_oh.ap(), oh[:, :, :TT].rearrange("p x t -> p (x t)"))
                nc.sync.dma_start(d_ohw.ap(), ohw[:, :, :TT].rearrange("p x t -> p (x t)"))
                dps_sb = res.tile([P, 2, BF], FP32)
                nc.vector.tensor_copy(dps_sb[:, 0, :], ps_s[:, 0, :])
                nc.vector.tensor_copy(dps_sb[:, 1, :], ps_w[:, 0, :])
                nc.sync.dma_start(d_ps.ap(), dps_sb[:].rearrange("p a f -> p (a f)"))
            aggT = psm.tile([P, F, K], BF16, tag="aggT")
            nc.vector.tensor_reduce(
                aggT[:, :, 2 * B:3 * B].transpose([0, 2, 1]),
                hmax[:].rearrange("p s (bb f) -> p (bb f) s", bb=B),
                axis=AX.X, op=OP.max, opt_input=False)
            # sym (a=0) / sum (a=1) from psum, transposed to [P, F, b]
            nc.scalar.copy(aggT[:, :, 0:B].transpose([0, 2, 1]),
                           ps_w[:, 0, :].rearrange("p (bb f) -> p bb f", bb=B))
            nc.scalar.copy(aggT[:, :, B:2 * B].transpose([0, 2, 1]),
                           ps_s[:, 0, :].rearrange("p (bb f) -> p bb f", bb=B))

            if DBG and b == 0:
                nc.sync.dma_start(d_aggT.ap(), aggT[:].rearrange("p f k -> p (f k)"))
            # einsum: tmp[p,h,f,k] = aggT[p,f,k] * comb[p,h,k]; tree-reduce k
            tmp = ptmp.tile([P, H, F, K], BF16, tag="tmp")
            nc.vector.tensor_tensor(
                tmp[:],
                aggT[:].unsqueeze(1).broadcast_to([P, H, F, K]),
                comb_sb[:, b, :].rearrange("p (h k) -> p h k", h=H)
                .unsqueeze(2).broadcast_to([P, H, F, K]),
                op=OP.mult)
            t6 = ptmp.tile([P, H, F, 6], BF16, tag="t6")
            nc.vector.tensor_tensor(t6[:], tmp[:, :, :, 0:6],
                                    tmp[:, :, :, 6:12], op=OP.add)
            t3 = ptmp.tile([P, H, F, 3], BF16, tag="t3")
            nc.vector.tensor_tensor(t3[:], t6[:, :, :, 0:3],
                                    t6[:, :, :, 3:6], op=OP.add)
            hbt = psm.tile([P, D], FP32, tag="hbt")
            nc.vector.tensor_reduce(hbt[:], t3[:], axis=AX.X, op=OP.add,
                                    opt_input=False)
            nc.vector.tensor_tensor(hb_all[:, b, :], hbt[:], cbias_sb[:],
                                    op=OP.add)
            hsq = psm.tile([P, D], BF16, tag="hsq")
            nc.scalar.square(hsq[:], hb_all[:, b, :])

            # graph one-hot + stats
            goh = goh_all[:, b, :]
            nc.vector.tensor_scalar(goh, iota_f[:, :G],
                                    gid_sb[:, b:b + 1], None, op0=OP.is_equal)
            nc.tensor.matmul(gsum_ps[:], goh, hb_all[:, b, :],
                             start=(b == 0), stop=(b == nblk - 1))
            nc.tensor.matmul(gsq_ps[:], goh, hsq[:],
                             start=(b == 0), stop=(b == nblk - 1))
            tb += TT

        if DBG:
            nc.sync.dma_start(d_tab0.ap(), bases0[:])
            nc.sync.dma_start(d_comb.ap(), comb_sb[:].rearrange("p b c -> p (b c)"))
            nc.sync.dma_start(d_hb.ap(), hb_all[:].rearrange("p b d -> p (b d)"))
        # ---- stage D: per-graph stats ------------------------------------
        stats = res.tile([G, 2, D], FP32)    # q' | rstd*gamma
        mean = pd.tile([G, D], FP32, tag="mean")
        nc.vector.tensor_scalar(mean[:], gsum_ps[:], invc_sb[:, 0:1], None,
                                op0=OP.mult)
        ex2 = pd.tile([G, D], FP32, tag="ex2")
        nc.vector.tensor_scalar(ex2[:], gsq_ps[:], invc_sb[:, 0:1], None,
                                op0=OP.mult)
        meansc = pd.tile([G, D], FP32, tag="meansc")
        nc.vector.tensor_tensor(meansc[:], mean[:], alphar_sb[:], op=OP.mult)
        t2 = pd.tile([G, D], FP32, tag="t2")
        nc.vector.scalar_tensor_tensor(t2[:], mean[:], 2.0, meansc[:],
                                       op0=OP.mult, op1=OP.subtract)
        var = pd.tile([G, D], FP32, tag="var")
        nc.vector.tensor_tensor(var[:], meansc[:], t2[:], op=OP.mult)
        nc.vector.tensor_tensor(var[:], ex2[:], var[:], op=OP.subtract)
        nc.vector.tensor_scalar(var[:], var[:], EPS, None, op0=OP.add)
        sd = pd.tile([G, D], FP32, tag="sd")
        nc.scalar.activation(sd[:], var[:], ACTF.Sqrt)
        rstd = pd.tile([G, D], FP32, tag="rstd")
        nc.vector.reciprocal(rstd[:], sd[:])
        nc.vector.tensor_tensor(stats[:, 1, :], rstd[:], gammar_sb[:],
                                op=OP.mult)
        # q = meansc * (rstd*gamma) - beta  -> out = h*s - q
        nc.vector.tensor_tensor(stats[:, 0, :], meansc[:],
                                stats[:, 1, :], op=OP.mult)
        nc.vector.tensor_tensor(stats[:, 0, :], stats[:, 0, :],
                                br_sb[:], op=OP.subtract)

        # ---- stage E: normalize + relu + out -----------------------------
        pagg_cm.__exit__(None, None, None)
        pacc_cm.__exit__(None, None, None)
        pe = ctx.enter_context(tc.tile_pool(name="pe", bufs=2, space="PSUM"))
        stats_bf_t = res.tile([G, 2, D], BF16)
        nc.vector.tensor_copy(stats_bf_t[:], stats[:])
        for b in range(nblk):
            gt_ps = pe.tile([G, P], BF16, tag="gt")
            nc.tensor.transpose(gt_ps[:], goh_all[:, b, :], ident_bf[:])
            gt = psm.tile([G, P], BF16, tag="gts")
            nc.scalar.copy(gt[:], gt_ps[:])
            stats_bf = stats_bf_t[:]
            bc = pe.tile([P, 2, D], FP32, tag="bc")
            nc.tensor.matmul(bc[:], gt[:], stats_bf, start=True, stop=True)
            hc = psm.tile([P, D], FP32, tag="hc")
            nc.vector.tensor_tensor(hc[:], hb_all[:, b, :], bc[:, 1, :],
                                    op=OP.mult)
            nc.vector.tensor_tensor(hc[:], hc[:], bc[:, 0, :], op=OP.subtract)
            ho = psm.tile([P, D], FP32, tag="ho")
            nc.scalar.activation(ho[:], hc[:], ACTF.Relu)
            nc.sync.dma_start(hout.ap()[b * P:(b + 1) * P, :], ho[:])

    return nc

# ======================= entry point =======================


def kernel(**inputs) -> np.ndarray:
    inputs = {k: np.asarray(v) for k, v in inputs.items()}
    lay = build(inputs["edge_index"].astype(np.int64),
                inputs["batch"].astype(np.int64))
    meta, in_maps = make_inputs(inputs, lay)

    nc = bacc.Bacc("TRN2", target_bir_lowering=False, debug=False,
                   num_devices=NCORES)
    build_program(nc, meta)
    nc.compile()
    res = bass_utils.run_bass_kernel_spmd(nc, in_maps,
                                          core_ids=list(range(NCORES)))
    outs = [res.results[c]["hout"] for c in range(NCORES)]
    kernel.last = dict(nc=nc, in_maps=in_maps, lay=lay, meta=meta)
    return unshard(lay, outs)



# revision 3
# speedup vs baseline: 3.4994x; 3.4994x over previous
"""EGConv layer (gnn_message_passing) on 8 Trainium2 NeuronCores.

Self-contained: kernel(**inputs) -> np.ndarray [50000, 256] float32.

Strategy: graph-aligned 1D node partition over 8 cores (GraphNorm fully
core-local), per-core degree-sorted node permutation, dst-sorted edge
streams. Each core computes the bases rows of only ITS nodes (plus comb
weights) from its local node shard; a device AllGather assembles the
full [8Q, BF] bf16 bases table on every core. Messages are fetched by
dma_gather from two int16-indexable halves of that table (split at the
core-aligned row 4Q); sum/sym aggregation via block-level one-hot
matmuls on the tensor engine; max via slot-layout gather + halve +
strided max-reduce; per-node einsum in bf16 on the vector engine;
GraphNorm via per-graph one-hot matmuls. Host->device traffic is
minimized: inputs packed into three dtype-blobs (bf16/int16/fp32),
gather-index streams shipped 16-row (replicated to 128 on device),
output returned in bf16. The SPMD program is identical across cores;
all per-core variation is in the data.
"""
import sys
for _p in ("/opt/trn_rl_repo", "/root/.axon_site/_ro/trn_rl_repo"):
    if _p not in sys.path:
        sys.path.insert(0, _p)

import os
import numpy as np
import ml_dtypes
from contextlib import ExitStack

import concourse.bass as bass
import concourse.mybir as mybir
import concourse.tile as tile
from concourse import bacc, bass_utils
from concourse.masks import make_identity

BFNP = ml_dtypes.bfloat16

# ======================= host-side graph preprocessing =======================

N, E, D = 50000, 800000, 256
H, B, A = 8, 4, 3
F = D // H          # 32
BF = B * F          # 128
G = 64
EPS = 1e-5
NCORES = 8
P = 128
NEG = -1e30


def build(edge_index: np.ndarray, batch: np.ndarray):
    """edge_index [2,E] int32, batch [N] int32 sorted. Returns layout dict."""
    src_all = np.concatenate([edge_index[0], np.arange(N, dtype=np.int64)])
    dst_all = np.concatenate([edge_index[1], np.arange(N, dtype=np.int64)])

    deg = np.bincount(dst_all, minlength=N).astype(np.float64)
    dinv = np.where(deg > 0, 1.0 / np.sqrt(deg), 0.0).astype(np.float32)
    symw_all = (dinv[src_all] * dinv[dst_all]).astype(np.float32)

    # graph-aligned 8-way shard
    gcnt = np.bincount(batch, minlength=G)
    gend = np.cumsum(gcnt)            # node index where graph g ends
    cuts = [0]
    for c in range(1, NCORES):
        target = round(N * c / NCORES)
        gi = np.argmin(np.abs(gend - target))
        cuts.append(int(gend[gi]))
    cuts.append(N)
    cuts = sorted(set(cuts))
    assert len(cuts) == NCORES + 1, cuts
    cuts_a = np.asarray(cuts, dtype=np.int64)
    src_core = np.searchsorted(cuts_a, src_all, side="right") - 1

    # pass 1: per-core degree-sorted permutation
    cores = []
    for c in range(NCORES):
        n0, n1 = cuts[c], cuts[c + 1]
        nloc = n1 - n0
        local_deg = deg[n0:n1]
        # secondary key: T0-range in-degree (srcs on cores 0-3), to tighten
        # per-range slot rectangles
        ldeg0 = np.bincount(dst_all[(dst_all >= n0) & (dst_all < n1)
                                    & (src_core < 4)] - n0,
                            minlength=nloc).astype(np.float64)
        perm = np.lexsort((-ldeg0, -local_deg)).astype(np.int64)  # desc
        gperm = perm + n0                      # new local id -> global id
        inv = np.empty(nloc, dtype=np.int64)
        inv[perm] = np.arange(nloc)            # orig local -> new local id
        cores.append(dict(n0=n0, n1=n1, nloc=nloc, gperm=gperm, inv=inv))

    maxloc = max(c["nloc"] for c in cores)
    Q = (maxloc // P + 1) * P                  # strictly > every nloc
    nblk = Q // P
    SPL = 4 * Q                                # T0/T1 split row (core-aligned)
    assert SPL - 1 <= 32767 and 4 * Q - 1 <= 32767

    ginv = np.empty(N, dtype=np.int64)         # orig global -> permuted row
    for c, core in enumerate(cores):
        ginv[core["gperm"]] = c * Q + np.arange(core["nloc"])

    # pass 2: per-core edge streams + global Tr/Sr
    nR = 2
    for core in cores:
        n0, n1 = core["n0"], core["n1"]
        emask = (dst_all >= n0) & (dst_all < n1)
        esrc = src_all[emask]
        edstl = core["inv"][dst_all[emask] - n0]   # new local dst id
        esym = symw_all[emask]
        order = np.argsort(edstl, kind="stable")
        core["esrc"], core["edstl"], core["esym"] = \
            esrc[order], edstl[order], esym[order]
        core["erow"] = ginv[core["esrc"]]          # permuted source row

    Tr = np.zeros((nR, nblk), dtype=np.int64)
    Sr = np.zeros((nR, nblk), dtype=np.int64)
    for c in cores:
        blk = c["edstl"] // P
        rng = (c["erow"] >= SPL).astype(np.int64)
        for r in range(nR):
            cnt = np.bincount(blk[rng == r], minlength=nblk)
            Tr[r] = np.maximum(Tr[r], (cnt + P - 1) // P)
            dl = c["edstl"][rng == r]
            nd = np.bincount(dl, minlength=nblk * P).reshape(nblk, P)
            Sr[r] = np.maximum(Sr[r], nd.max(axis=1))
    Tr = np.maximum(Tr, 1)
    Sr = np.maximum(Sr, 1)

    PAD0, PAD1 = SPL - 1, 4 * Q - 1   # NEG tail rows (cores 3 / 7), per-range

    sumTT = int((Tr[0] + Tr[1]).sum())
    for c in cores:
        nloc = c["nloc"]
        dstl_t = np.full((P, sumTT), -1.0, dtype=BFNP)
        symw_t = np.zeros((P, sumTT), dtype=BFNP)
        flat_r = [[], []]
        blk = c["edstl"] // P
        rng = (c["erow"] >= SPL).astype(np.int64)
        tcol = 0
        for b in range(nblk):
            for r in range(nR):
                m = (blk == b) & (rng == r)
                srow = c["erow"][m] - (SPL if r else 0)
                dl = c["edstl"][m] - b * P
                sw = c["esym"][m]
                k = len(srow)
                T, S = int(Tr[r][b]), int(Sr[r][b])
                pad = PAD1 if r else PAD0
                ef = np.full(P * T, pad, dtype=np.int64)
                ef[:k] = srow
                flat_r[r].append(ef)
                cols = tcol + np.arange(k) // P
                rows = np.arange(k) % P
                dstl_t[rows, cols] = dl.astype(np.float32)
                symw_t[rows, cols] = sw
                tcol += T
                sf = np.full(P * S, pad, dtype=np.int64)
                if k:
                    marks = np.flatnonzero(np.diff(dl, prepend=-1))
                    slot = np.arange(k) - np.repeat(marks, np.diff(
                        np.append(marks, k)))
                    sf[slot * P + dl] = srow
                    # pad slots of nodes that HAVE >=1 edge in this range:
                    # duplicate the node's first edge (max unchanged, avoids
                    # a NEG-row fetch)
                    first = np.full(P, -1, dtype=np.int64)
                    first[dl[marks]] = srow[marks]
                    degr = np.zeros(P, dtype=np.int64)
                    dcnt = np.diff(np.append(marks, k))
                    degr[dl[marks]] = dcnt
                    for s in range(S):
                        lane = np.flatnonzero((degr > 0) & (degr <= s))
                        sf[s * P + lane] = first[lane]
                flat_r[r].append(sf)
        i16 = []
        for r in range(nR):
            fl = np.concatenate(flat_r[r])
            L = len(fl)
            w = np.zeros((16, L // 16), dtype=np.int16)
            w[np.arange(L) % 16, np.arange(L) // 16] = fl
            i16.append(w)

        gl0 = batch[c["n0"]]
        ngid = np.full(nblk * P, -1.0, dtype=np.float32)
        ngid[:nloc] = (batch[c["gperm"]] - gl0).astype(np.float32)
        gid_t = ngid.reshape(nblk, P).T.copy()

        nmv = np.zeros(nblk * P, dtype=np.float32)
        nmv[nloc:] = NEG
        nmask_t = nmv.reshape(nblk, P).T.copy()

        icnt = np.ones((G, 1), dtype=np.float32)
        glo = np.bincount((batch[c["n0"]:c["n1"]] - gl0), minlength=G)
        icnt[glo > 0, 0] = (1.0 / glo[glo > 0]).astype(np.float32)
        invc = np.ones((G, 1), dtype=np.float32)
        invc[:icnt.shape[0]] = icnt

        c.update(dstl_t=dstl_t, symw_t=symw_t, i16_0=i16[0], i16_1=i16[1],
                 gid_t=gid_t, nmask_t=nmask_t, invcnt=invc)

    return dict(cores=cores, nblk=nblk, Q=Q, Tr=Tr, Sr=Sr, cuts=cuts)


def unshard(layout, per_core_out):
    full = np.zeros((N, D), dtype=np.float32)
    for c, out in zip(layout["cores"], per_core_out):
        full[c["gperm"]] = out[:c["nloc"]].astype(np.float32)
    return full

# ============ input-map construction ============


def to_bf16(x):
    return np.asarray(x, np.float32).astype(BFNP)


def make_inputs(inputs, lay):
    """inputs: dict of full np arrays. lay: build output.
    Returns (meta, in_maps list of 8 dicts)."""
    Q = lay["Q"]
    nblk = lay["nblk"]

    node = np.asarray(inputs["node"], np.float32)
    wb = to_bf16(inputs["W_bases"])                       # [D, BF]
    wc = to_bf16(inputs["W_comb"])                        # [D, HBA]
    bcomb = np.tile(np.asarray(inputs["b_comb"], np.float32)[None, :], (P, 1))
    cbias = np.tile(np.asarray(inputs["conv_bias"], np.float32)[None, :], (P, 1))
    alphar = np.tile(np.asarray(inputs["gn_mean_scale"], np.float32)[None, :], (G, 1))
    gammar = np.tile(np.asarray(inputs["gn_weight"], np.float32)[None, :], (G, 1))
    br = np.tile(np.asarray(inputs["gn_bias"], np.float32)[None, :], (G, 1))

    meta = dict(Q=Q, nblk=nblk,
                Tr0=[int(x) for x in lay["Tr"][0]],
                Tr1=[int(x) for x in lay["Tr"][1]],
                Sr0=[int(x) for x in lay["Sr"][0]],
                Sr1=[int(x) for x in lay["Sr"][1]])

    in_maps = []
    for c in lay["cores"]:
        ntl = np.zeros((D, Q), BFNP)
        ntl[:, :c["nloc"]] = to_bf16(node[c["gperm"]]).T
        blob_bf = np.concatenate([
            ntl.ravel(), wb.ravel(), wc.ravel(),
            c["dstl_t"].ravel(), c["symw_t"].ravel()])
        blob_i16 = np.concatenate([c["i16_0"].ravel(), c["i16_1"].ravel()])
        blob_f32 = np.concatenate([
            bcomb.ravel(), c["gid_t"].ravel(), c["invcnt"].ravel(),
            alphar.ravel(), gammar.ravel(), br.ravel(), cbias.ravel(),
            c["nmask_t"].ravel()])
        in_maps.append(dict(blob_bf=blob_bf, blob_i16=blob_i16,
                            blob_f32=blob_f32))
    return meta, in_maps

# ============ device program ============

FP32 = mybir.dt.float32
BF16 = mybir.dt.bfloat16
I32 = mybir.dt.int32
I16 = mybir.dt.int16
AX = mybir.AxisListType
OP = mybir.AluOpType
ACTF = mybir.ActivationFunctionType
HBA = H * B * A   # 96
K = B * A         # 12


def build_program(nc, meta):
    Q = meta["Q"]
    nblk = meta["nblk"]
    Tr0, Tr1 = list(meta["Tr0"]), list(meta["Tr1"])
    Sr0, Sr1 = list(meta["Sr0"]), list(meta["Sr1"])
    sumT = sum(Tr0) + sum(Tr1)
    L0 = sum(8 * (t + s) for t, s in zip(Tr0, Sr0))   # cols of [16, L0]
    L1 = sum(8 * (t + s) for t, s in zip(Tr1, Sr1))
    TTmax = max(t0 + t1 for t0, t1 in zip(Tr0, Tr1))
    SPL = 4 * Q

    # ---- external blobs ---------------------------------------------------
    LBF = D * Q + D * BF + D * HBA + 2 * P * sumT
    LF32 = P * HBA + P * nblk + G + 3 * G * D + P * D + P * nblk
    blob_bf = nc.dram_tensor("blob_bf", [LBF], BF16, kind="ExternalInput")
    blob_i16 = nc.dram_tensor("blob_i16", [16 * (L0 + L1)], I16,
                              kind="ExternalInput")
    blob_f32 = nc.dram_tensor("blob_f32", [LF32], FP32, kind="ExternalInput")
    hout = nc.dram_tensor("hout", [Q, D], BF16, kind="ExternalOutput")

    o_ntl = 0
    o_wb = o_ntl + D * Q
    o_wc = o_wb + D * BF
    o_dstl = o_wc + D * HBA
    o_symw = o_dstl + P * sumT
    f_bcomb = 0
    f_gid = f_bcomb + P * HBA
    f_invc = f_gid + P * nblk
    f_alphar = f_invc + G
    f_gammar = f_alphar + G * D
    f_br = f_gammar + G * D
    f_cbias = f_br + G * D
    f_nmask = f_cbias + P * D

    with ExitStack() as ctx:
        tc = ctx.enter_context(tile.TileContext(nc))
        dram = ctx.enter_context(tc.tile_pool(name="dram", bufs=1, space="DRAM"))
        res = ctx.enter_context(tc.tile_pool(name="res", bufs=1))
        pa = ctx.enter_context(tc.tile_pool(name="pa", bufs=3))
        pgath = ctx.enter_context(tc.tile_pool(name="pgath", bufs=2))
        pidx = ctx.enter_context(tc.tile_pool(name="pidx", bufs=2))
        poh = ctx.enter_context(tc.tile_pool(name="poh", bufs=2))
        ptmp = ctx.enter_context(tc.tile_pool(name="ptmp", bufs=2))
        psm = ctx.enter_context(tc.tile_pool(name="psm", bufs=2))
        pd = ctx.enter_context(tc.tile_pool(name="pd", bufs=1))

        # ---- constants / resident tiles ----------------------------------
        wb_sb = res.tile([P, 2, BF], BF16)
        nc.sync.dma_start(wb_sb[:], blob_bf.ap()[o_wb:o_wb + D * BF]
                          .rearrange("(a p f) -> p a f", p=P, f=BF))
        wc_sb = res.tile([P, 2, HBA], BF16)
        nc.sync.dma_start(wc_sb[:], blob_bf.ap()[o_wc:o_wc + D * HBA]
                          .rearrange("(a p f) -> p a f", p=P, f=HBA))
        dstl_sb = res.tile([P, sumT], BF16)
        nc.sync.dma_start(dstl_sb[:], blob_bf.ap()[o_dstl:o_dstl + P * sumT]
                          .rearrange("(p t) -> p t", p=P))
        symw_bf = res.tile([P, sumT], BF16)
        nc.sync.dma_start(symw_bf[:], blob_bf.ap()[o_symw:o_symw + P * sumT]
                          .rearrange("(p t) -> p t", p=P))
        symw_sb = res.tile([P, sumT], FP32)   # scalar-engine scale must be FP32
        nc.vector.tensor_copy(symw_sb[:], symw_bf[:])
        bcomb_sb = res.tile([P, HBA], FP32)
        nc.sync.dma_start(bcomb_sb[:], blob_f32.ap()[f_bcomb:f_bcomb + P * HBA]
                          .rearrange("(p t) -> p t", p=P))
        gid_sb = res.tile([P, nblk], FP32)
        nc.sync.dma_start(gid_sb[:], blob_f32.ap()[f_gid:f_gid + P * nblk]
                          .rearrange("(p t) -> p t", p=P))
        invc_sb = res.tile([G, 1], FP32)
        nc.sync.dma_start(invc_sb[:], blob_f32.ap()[f_invc:f_invc + G]
                          .rearrange("(p t) -> p t", p=G))
        alphar_sb = res.tile([G, D], FP32)
        nc.sync.dma_start(alphar_sb[:], blob_f32.ap()[f_alphar:f_alphar + G * D]
                          .rearrange("(p t) -> p t", p=G))
        gammar_sb = res.tile([G, D], FP32)
        nc.sync.dma_start(gammar_sb[:], blob_f32.ap()[f_gammar:f_gammar + G * D]
                          .rearrange("(p t) -> p t", p=G))
        br_sb = res.tile([G, D], FP32)
        nc.sync.dma_start(br_sb[:], blob_f32.ap()[f_br:f_br + G * D]
                          .rearrange("(p t) -> p t", p=G))
        cbias_sb = res.tile([P, D], FP32)
        nc.sync.dma_start(cbias_sb[:], blob_f32.ap()[f_cbias:f_cbias + P * D]
                          .rearrange("(p t) -> p t", p=P))
        nmask_sb = res.tile([P, nblk], FP32)
        nc.sync.dma_start(nmask_sb[:], blob_f32.ap()[f_nmask:f_nmask + P * nblk]
                          .rearrange("(p t) -> p t", p=P))

        ident = res.tile([P, P], FP32)
        make_identity(nc, ident[:])
        ident_bf = res.tile([P, P], BF16)
        nc.vector.tensor_copy(ident_bf[:], ident[:])
        iota_i = res.tile([P, P], I32)
        nc.gpsimd.iota(iota_i[:], pattern=[[1, P]], base=0, channel_multiplier=0)
        iota_f = res.tile([P, P], FP32)
        nc.vector.tensor_copy(iota_f[:], iota_i[:])
        iota_bf = res.tile([P, P], BF16)
        nc.vector.tensor_copy(iota_bf[:], iota_i[:])
        iota_exp = res.tile([P, P, TTmax], BF16)
        nc.scalar.copy(iota_exp[:],
                       iota_bf[:].unsqueeze(2).broadcast_to([P, P, TTmax]))

        comb_sb = res.tile([P, nblk, HBA], BF16)
        goh_all = res.tile([P, nblk, G], BF16)
        hb_all = res.tile([P, nblk, D], BF16)

        # ---- 16->128 replication of gather-index streams ------------------
        rep0 = dram.tile([P, L0], I16)
        rep1 = dram.tile([P, L1], I16)
        for kk in range(8):
            nc.sync.dma_start(rep0[16 * kk:16 * (kk + 1), :],
                              blob_i16.ap()[0:16 * L0]
                              .rearrange("(a l) -> a l", a=16))
            nc.sync.dma_start(rep1[16 * kk:16 * (kk + 1), :],
                              blob_i16.ap()[16 * L0:16 * (L0 + L1)]
                              .rearrange("(a l) -> a l", a=16))

        # ---- stage A: local bases segment + comb, then AllGather ----------
        mybases = dram.tile([Q, BF], BF16)
        bases_all = dram.tile([NCORES * Q, BF], BF16)
        pab_cm = tc.tile_pool(name="pab", bufs=4, space="PSUM")
        pab = pab_cm.__enter__()
        pcb_cm = tc.tile_pool(name="pcb", bufs=2, space="PSUM")
        pcb = pcb_cm.__enter__()

        ntl_ap = blob_bf.ap()[o_ntl:o_ntl + D * Q].rearrange(
            "(a p n) -> p a n", p=P, n=Q)
        for b in range(nblk):
            lt2 = pa.tile([P, 2, P], BF16, tag="lt")
            nc.sync.dma_start(lt2[:], ntl_ap[:, :, b * P:(b + 1) * P])
            ps = pab.tile([P, BF], FP32, tag="ab")
            nc.tensor.matmul(ps[:], lt2[:, 0, :], wb_sb[:, 0, :],
                             start=True, stop=False)
            nc.tensor.matmul(ps[:], lt2[:, 1, :], wb_sb[:, 1, :],
                             start=False, stop=True)
            ob = pa.tile([P, BF], BF16, tag="ob")
            nc.vector.tensor_scalar(ob[:], ps[:], nmask_sb[:, b:b + 1], None,
                                    op0=OP.add)
            nc.sync.dma_start(mybases[b * P:(b + 1) * P, :], ob[:])
            cps = pcb.tile([P, HBA], FP32, tag="cps")
            nc.tensor.matmul(cps[:], lt2[:, 0, :], wc_sb[:, 0, :],
                             start=True, stop=False)
            nc.tensor.matmul(cps[:], lt2[:, 1, :], wc_sb[:, 1, :],
                             start=False, stop=True)
            nc.vector.tensor_tensor(comb_sb[:, b, :], cps[:], bcomb_sb[:],
                                    op=OP.add)

        pcb_cm.__exit__(None, None, None)
        pab_cm.__exit__(None, None, None)

        nc.gpsimd.collective_compute(
            "AllGather", OP.bypass,
            replica_groups=[list(range(NCORES))],
            ins=[mybases[:].opt()],
            outs=[bases_all[:].opt()])

        # ---- stage C: gather + aggregate + einsum + stats -----------------
        pacc_cm = tc.tile_pool(name="pacc", bufs=1, space="PSUM")
        pacc = pacc_cm.__enter__()
        pagg_cm = tc.tile_pool(name="pagg", bufs=2, space="PSUM")
        pagg = pagg_cm.__enter__()
        gsum_ps = pacc.tile([G, D], FP32)
        gsq_ps = pacc.tile([G, D], FP32)

        CH = 64                       # <=8192 idxs per dma_gather call
        c0 = 0
        c1 = 0
        tb = 0
        for b in range(nblk):
            T0, T1 = Tr0[b], Tr1[b]
            S0, S1 = Sr0[b], Sr1[b]
            W0, W1 = T0 + S0, T1 + S1
            TT = T0 + T1
            gath = pgath.tile([P, W0 + W1, BF], BF16, tag="gath")
            if b < 2:
                nc.gpsimd.memset(gath[:], 0.0)
            ix0 = pidx.tile([P, 8 * W0], I16, tag="ix0")
            nc.sync.dma_start(ix0[:], rep0[:, c0:c0 + 8 * W0])
            ix1 = pidx.tile([P, 8 * W1], I16, tag="ix1")
            nc.sync.dma_start(ix1[:], rep1[:, c1:c1 + 8 * W1])
            for w0 in range(0, W0, CH):
                w = min(CH, W0 - w0)
                nc.gpsimd.dma_gather(
                    out_ap=gath[:, w0:w0 + w, :], in_ap=bases_all[0:SPL, :],
                    idxs_ap=ix0[:, 8 * w0:8 * (w0 + w)],
                    num_idxs=P * w, num_idxs_reg=P * w, elem_size=BF,
                    single_packet=False)
            for w1 in range(0, W1, CH):
                w = min(CH, W1 - w1)
                nc.gpsimd.dma_gather(
                    out_ap=gath[:, W0 + w1:W0 + w1 + w, :],
                    in_ap=bases_all[SPL:NCORES * Q, :],
                    idxs_ap=ix1[:, 8 * w1:8 * (w1 + w)],
                    num_idxs=P * w, num_idxs_reg=P * w, elem_size=BF,
                    single_packet=False)
            c0 += 8 * W0
            c1 += 8 * W1

            # block-level one-hot builds: oh[p_edge, x, t]
            oh = poh.tile([P, P, TTmax], BF16, tag="oh")
            nc.vector.tensor_tensor(
                oh[:, :, :TT],
                dstl_sb[:, tb:tb + TT].unsqueeze(1).broadcast_to([P, P, TT]),
                iota_exp[:, :, :TT], op=OP.is_equal)
            ohw = poh.tile([P, P, TTmax], BF16, tag="ohw")
            for t in range(TT):
                nc.scalar.mul(ohw[:, :, t], oh[:, :, t],
                              symw_sb[:, tb + t:tb + t + 1])

            ps_s = pagg.tile([P, 4, BF], FP32, tag="aggsum")
            ps_w = pagg.tile([P, 4, BF], FP32, tag="aggsym")
            for t in range(TT):
                mcol = t if t < T0 else W0 + (t - T0)
                nc.tensor.matmul(ps_s[:, 0, :], oh[:, :, t], gath[:, mcol, :],
                                 start=(t == 0), stop=(t == TT - 1))
                nc.tensor.matmul(ps_w[:, 0, :], ohw[:, :, t], gath[:, mcol, :],
                                 start=(t == 0), stop=(t == TT - 1))

            # max: halve (overlap-safe) then one strided reduce over both
            # ranges into aggT[:, :, 8:12]
            m0, m1 = (S0 + 1) // 2, (S1 + 1) // 2
            hmax = ptmp.tile([P, m0 + m1, BF], BF16, tag="hmax")
            nc.vector.tensor_tensor(hmax[:, :m0, :],
                                    gath[:, T0:T0 + m0, :],
                                    gath[:, T0 + S0 - m0:T0 + S0, :],
                                    op=OP.max)
            nc.vector.tensor_tensor(hmax[:, m0:m0 + m1, :],
                                    gath[:, W0 + T1:W0 + T1 + m1, :],
                                    gath[:, W0 + W1 - m1:W0 + W1, :],
                                    op=OP.max)
            aggT = psm.tile([P, F, K], BF16, tag="aggT")
            nc.vector.tensor_reduce(
                aggT[:, :, 2 * B:3 * B].transpose([0, 2, 1]),
                hmax[:].rearrange("p s (bb f) -> p (bb f) s", bb=B),
                axis=AX.X, op=OP.max, opt_input=False)
            # sym (a=0) / sum (a=1) from psum, transposed to [P, F, b]
            nc.scalar.copy(aggT[:, :, 0:B].transpose([0, 2, 1]),
                           ps_w[:, 0, :].rearrange("p (bb f) -> p bb f", bb=B))
            nc.scalar.copy(aggT[:, :, B:2 * B].transpose([0, 2, 1]),
                           ps_s[:, 0, :].rearrange("p (bb f) -> p bb f", bb=B))

            # einsum: tmp[p,h,f,k] = aggT[p,f,k] * comb[p,h,k]; tree-reduce k
            tmp = ptmp.tile([P, H, F, K], BF16, tag="tmp")
            nc.vector.tensor_tensor(
                tmp[:],
                aggT[:].unsqueeze(1).broadcast_to([P, H, F, K]),
                comb_sb[:, b, :].rearrange("p (h k) -> p h k", h=H)
                .unsqueeze(2).broadcast_to([P, H, F, K]),
                op=OP.mult)
            t6 = ptmp.tile([P, H, F, 6], BF16, tag="t6")
            nc.vector.tensor_tensor(t6[:], tmp[:, :, :, 0:6],
                                    tmp[:, :, :, 6:12], op=OP.add)
            t3 = ptmp.tile([P, H, F, 3], BF16, tag="t3")
            nc.vector.tensor_tensor(t3[:], t6[:, :, :, 0:3],
                                    t6[:, :, :, 3:6], op=OP.add)
            hbt = psm.tile([P, D], FP32, tag="hbt")
            nc.vector.tensor_reduce(hbt[:], t3[:], axis=AX.X, op=OP.add,
                                    opt_input=False)
            nc.vector.tensor_tensor(hb_all[:, b, :], hbt[:], cbias_sb[:],
                                    op=OP.add)
            hsq = psm.tile([P, D], BF16, tag="hsq")
            nc.scalar.square(hsq[:], hb_all[:, b, :])

            # graph one-hot + stats
            goh = goh_all[:, b, :]
            nc.vector.tensor_scalar(goh, iota_f[:, :G],
                                    gid_sb[:, b:b + 1], None, op0=OP.is_equal)
            nc.tensor.matmul(gsum_ps[:], goh, hb_all[:, b, :],
                             start=(b == 0), stop=(b == nblk - 1))
            nc.tensor.matmul(gsq_ps[:], goh, hsq[:],
                             start=(b == 0), stop=(b == nblk - 1))
            tb += TT

        # ---- stage D: per-graph stats ------------------------------------
        stats = res.tile([G, 2, D], FP32)    # q' | rstd*gamma
        mean = pd.tile([G, D], FP32, tag="mean")
        nc.vector.tensor_scalar(mean[:], gsum_ps[:], invc_sb[:, 0:1], None,
                                op0=OP.mult)
        ex2 = pd.tile([G, D], FP32, tag="ex2")
        nc.vector.tensor_scalar(ex2[:], gsq_ps[:], invc_sb[:, 0:1], None,
                                op0=OP.mult)
        meansc = pd.tile([G, D], FP32, tag="meansc")
        nc.vector.tensor_tensor(meansc[:], mean[:], alphar_sb[:], op=OP.mult)
        t2 = pd.tile([G, D], FP32, tag="t2")
        nc.vector.scalar_tensor_tensor(t2[:], mean[:], 2.0, meansc[:],
                                       op0=OP.mult, op1=OP.subtract)
        var = pd.tile([G, D], FP32, tag="var")
        nc.vector.tensor_tensor(var[:], meansc[:], t2[:], op=OP.mult)
        nc.vector.tensor_tensor(var[:], ex2[:], var[:], op=OP.subtract)
        nc.vector.tensor_scalar(var[:], var[:], EPS, None, op0=OP.add)
        sd = pd.tile([G, D], FP32, tag="sd")
        nc.scalar.activation(sd[:], var[:], ACTF.Sqrt)
        rstd = pd.tile([G, D], FP32, tag="rstd")
        nc.vector.reciprocal(rstd[:], sd[:])
        nc.vector.tensor_tensor(stats[:, 1, :], rstd[:], gammar_sb[:],
                                op=OP.mult)
        # q = meansc * (rstd*gamma) - beta  -> out = h*s - q
        nc.vector.tensor_tensor(stats[:, 0, :], meansc[:],
                                stats[:, 1, :], op=OP.mult)
        nc.vector.tensor_tensor(stats[:, 0, :], stats[:, 0, :],
                                br_sb[:], op=OP.subtract)

        # ---- stage E: normalize + relu + out -----------------------------
        pagg_cm.__exit__(None, None, None)
        pacc_cm.__exit__(None, None, None)
        pe = ctx.enter_context(tc.tile_pool(name="pe", bufs=2, space="PSUM"))
        stats_bf_t = res.tile([G, 2, D], BF16)
        nc.vector.tensor_copy(stats_bf_t[:], stats[:])
        for b in range(nblk):
            gt_ps = pe.tile([G, P], BF16, tag="gt")
            nc.tensor.transpose(gt_ps[:], goh_all[:, b, :], ident_bf[:])
            gt = psm.tile([G, P], BF16, tag="gts")
            nc.scalar.copy(gt[:], gt_ps[:])
            stats_bf = stats_bf_t[:]
            bc = pe.tile([P, 2, D], FP32, tag="bc")
            nc.tensor.matmul(bc[:], gt[:], stats_bf, start=True, stop=True)
            hc = psm.tile([P, D], FP32, tag="hc")
            nc.vector.tensor_tensor(hc[:], hb_all[:, b, :], bc[:, 1, :],
                                    op=OP.mult)
            nc.vector.tensor_tensor(hc[:], hc[:], bc[:, 0, :], op=OP.subtract)
            ho = psm.tile([P, D], BF16, tag="ho")
            nc.scalar.activation(ho[:], hc[:], ACTF.Relu)
            nc.sync.dma_start(hout.ap()[b * P:(b + 1) * P, :], ho[:])

    return nc

# ======================= entry point =======================


def kernel(**inputs) -> np.ndarray:
    inputs = {k: np.asarray(v) for k, v in inputs.items()}
    lay = build(inputs["edge_index"].astype(np.int64),
                inputs["batch"].astype(np.int64))
    meta, in_maps = make_inputs(inputs, lay)

    nc = bacc.Bacc("TRN2", target_bir_lowering=False, debug=False,
                   num_devices=NCORES)
    build_program(nc, meta)
    nc.compile()
    res = bass_utils.run_bass_kernel_spmd(nc, in_maps,
                                          core_ids=list(range(NCORES)))
    outs = [res.results[c]["hout"] for c in range(NCORES)]
    kernel.last = dict(nc=nc, in_maps=in_maps, lay=lay, meta=meta)
    return unshard(lay, outs)


# revision 6
# speedup vs baseline: 5.3332x; 1.5240x over previous
"""EGConv layer (gnn_message_passing) on 8 Trainium2 NeuronCores.

Self-contained: kernel(**inputs) -> np.ndarray [50000, 256] float32.

Strategy: graph-aligned 1D node partition over 8 cores (GraphNorm fully
core-local), per-core degree-sorted node permutation, dst-sorted edge
streams. Each core computes the bases rows of only ITS nodes (plus comb
weights) from its local node shard; a device AllGather assembles the
full [8Q, BF] bf16 bases table on every core. Messages are fetched by
dma_gather from two int16-indexable halves of that table (split at the
core-aligned row 4Q); sum/sym aggregation via block-level one-hot
matmuls on the tensor engine; max via slot-layout gather + halve +
strided max-reduce; per-node einsum in bf16 on the vector engine;
GraphNorm via per-graph one-hot matmuls. Host->device traffic is
minimized: inputs packed into three dtype-blobs (bf16/int16/fp32),
gather-index streams shipped 16-row (replicated to 128 on device),
output returned in bf16. The SPMD program is identical across cores;
all per-core variation is in the data.
"""
import sys
for _p in ("/opt/trn_rl_repo", "/root/.axon_site/_ro/trn_rl_repo"):
    if _p not in sys.path:
        sys.path.insert(0, _p)

import os
import numpy as np
import ml_dtypes
from contextlib import ExitStack

import concourse.bass as bass
import concourse.mybir as mybir
import concourse.tile as tile
from concourse import bacc, bass_utils
from concourse.masks import make_identity

BFNP = ml_dtypes.bfloat16

# ======================= host-side graph preprocessing =======================

N, E, D = 50000, 800000, 256
H, B, A = 8, 4, 3
F = D // H          # 32
BF = B * F          # 128
G = 64
EPS = 1e-5
NCORES = 8
P = 128
NEG = -1e30


def build(edge_index: np.ndarray, batch: np.ndarray):
    """edge_index [2,E] int32, batch [N] int32 sorted. Returns layout dict."""
    src_all = np.concatenate([edge_index[0], np.arange(N, dtype=np.int64)])
    dst_all = np.concatenate([edge_index[1], np.arange(N, dtype=np.int64)])

    deg = np.bincount(dst_all, minlength=N).astype(np.float64)
    dinv = np.where(deg > 0, 1.0 / np.sqrt(deg), 0.0).astype(np.float32)
    symw_all = (dinv[src_all] * dinv[dst_all]).astype(np.float32)

    # graph-aligned 8-way shard
    gcnt = np.bincount(batch, minlength=G)
    gend = np.cumsum(gcnt)            # node index where graph g ends
    cuts = [0]
    for c in range(1, NCORES):
        target = round(N * c / NCORES)
        gi = np.argmin(np.abs(gend - target))
        cuts.append(int(gend[gi]))
    cuts.append(N)
    cuts = sorted(set(cuts))
    assert len(cuts) == NCORES + 1, cuts
    cuts_a = np.asarray(cuts, dtype=np.int64)
    src_core = np.searchsorted(cuts_a, src_all, side="right") - 1

    # pass 1: per-core degree-sorted permutation
    cores = []
    for c in range(NCORES):
        n0, n1 = cuts[c], cuts[c + 1]
        nloc = n1 - n0
        local_deg = deg[n0:n1]
        # secondary key: T0-range in-degree (srcs on cores 0-3), to tighten
        # per-range slot rectangles
        ldeg0 = np.bincount(dst_all[(dst_all >= n0) & (dst_all < n1)
                                    & (src_core < 4)] - n0,
                            minlength=nloc).astype(np.float64)
        perm = np.lexsort((-ldeg0, -local_deg)).astype(np.int64)  # desc
        gperm = perm + n0                      # new local id -> global id
        inv = np.empty(nloc, dtype=np.int64)
        inv[perm] = np.arange(nloc)            # orig local -> new local id
        cores.append(dict(n0=n0, n1=n1, nloc=nloc, gperm=gperm, inv=inv))

    maxloc = max(c["nloc"] for c in cores)
    Q = (maxloc // P + 1) * P                  # strictly > every nloc
    nblk = Q // P
    SPL = 4 * Q                                # T0/T1 split row (core-aligned)
    assert SPL - 1 <= 32767 and 4 * Q - 1 <= 32767

    ginv = np.empty(N, dtype=np.int64)         # orig global -> permuted row
    for c, core in enumerate(cores):
        ginv[core["gperm"]] = c * Q + np.arange(core["nloc"])

    # pass 2: per-core edge streams + global Tr/Sr
    nR = 2
    for core in cores:
        n0, n1 = core["n0"], core["n1"]
        emask = (dst_all >= n0) & (dst_all < n1)
        esrc = src_all[emask]
        edstl = core["inv"][dst_all[emask] - n0]   # new local dst id
        esym = symw_all[emask]
        order = np.argsort(edstl, kind="stable")
        core["esrc"], core["edstl"], core["esym"] = \
            esrc[order], edstl[order], esym[order]
        core["erow"] = ginv[core["esrc"]]          # permuted source row

    Tr = np.zeros((nR, nblk), dtype=np.int64)
    Sr = np.zeros((nR, nblk), dtype=np.int64)
    for c in cores:
        blk = c["edstl"] // P
        rng = (c["erow"] >= SPL).astype(np.int64)
        for r in range(nR):
            cnt = np.bincount(blk[rng == r], minlength=nblk)
            Tr[r] = np.maximum(Tr[r], (cnt + P - 1) // P)
            dl = c["edstl"][rng == r]
            nd = np.bincount(dl, minlength=nblk * P).reshape(nblk, P)
            Sr[r] = np.maximum(Sr[r], nd.max(axis=1))
    Tr = np.maximum(Tr, 1)
    Sr = np.maximum(Sr, 1)

    PAD0, PAD1 = SPL - 1, 4 * Q - 1   # NEG tail rows (cores 3 / 7), per-range

    sumTT = int((Tr[0] + Tr[1]).sum())
    for c in cores:
        nloc = c["nloc"]
        dstl_t = np.full((P, sumTT), -1.0, dtype=BFNP)
        symw_t = np.zeros((P, sumTT), dtype=BFNP)
        flat_r = [[], []]
        blk = c["edstl"] // P
        rng = (c["erow"] >= SPL).astype(np.int64)
        tcol = 0
        for b in range(nblk):
            for r in range(nR):
                m = (blk == b) & (rng == r)
                srow = c["erow"][m] - (SPL if r else 0)
                dl = c["edstl"][m] - b * P
                sw = c["esym"][m]
                k = len(srow)
                T, S = int(Tr[r][b]), int(Sr[r][b])
                pad = PAD1 if r else PAD0
                ef = np.full(P * T, pad, dtype=np.int64)
                ef[:k] = srow
                flat_r[r].append(ef)
                cols = tcol + np.arange(k) // P
                rows = np.arange(k) % P
                dstl_t[rows, cols] = dl.astype(np.float32)
                symw_t[rows, cols] = sw
                tcol += T
                sf = np.full(P * S, pad, dtype=np.int64)
                if k:
                    marks = np.flatnonzero(np.diff(dl, prepend=-1))
                    slot = np.arange(k) - np.repeat(marks, np.diff(
                        np.append(marks, k)))
                    sf[slot * P + dl] = srow
                    # pad slots of nodes that HAVE >=1 edge in this range:
                    # duplicate the node's first edge (max unchanged, avoids
                    # a NEG-row fetch)
                    first = np.full(P, -1, dtype=np.int64)
                    first[dl[marks]] = srow[marks]
                    degr = np.zeros(P, dtype=np.int64)
                    dcnt = np.diff(np.append(marks, k))
                    degr[dl[marks]] = dcnt
                    for s in range(S):
                        lane = np.flatnonzero((degr > 0) & (degr <= s))
                        sf[s * P + lane] = first[lane]
                flat_r[r].append(sf)
        i16 = []
        for r in range(nR):
            fl = np.concatenate(flat_r[r])
            L = len(fl)
            w = np.zeros((16, L // 16), dtype=np.int16)
            w[np.arange(L) % 16, np.arange(L) // 16] = fl
            i16.append(w)

        gl0 = batch[c["n0"]]
        ngid = np.full(nblk * P, -1.0, dtype=np.float32)
        ngid[:nloc] = (batch[c["gperm"]] - gl0).astype(np.float32)
        gid_t = ngid.reshape(nblk, P).T.copy()

        nmv = np.zeros(nblk * P, dtype=np.float32)
        nmv[nloc:] = NEG
        nmask_t = nmv.reshape(nblk, P).T.copy()

        icnt = np.ones((G, 1), dtype=np.float32)
        glo = np.bincount((batch[c["n0"]:c["n1"]] - gl0), minlength=G)
        icnt[glo > 0, 0] = (1.0 / glo[glo > 0]).astype(np.float32)
        invc = np.ones((G, 1), dtype=np.float32)
        invc[:icnt.shape[0]] = icnt

        c.update(dstl_t=dstl_t, symw_t=symw_t, i16_0=i16[0], i16_1=i16[1],
                 gid_t=gid_t, nmask_t=nmask_t, invcnt=invc)

    return dict(cores=cores, nblk=nblk, Q=Q, Tr=Tr, Sr=Sr, cuts=cuts)


def unshard(layout, per_core_out):
    full = np.zeros((N, D), dtype=np.float32)
    for c, out in zip(layout["cores"], per_core_out):
        full[c["gperm"]] = out[:c["nloc"]].astype(np.float32)
    return full

# ============ input-map construction ============


def to_bf16(x):
    return np.asarray(x, np.float32).astype(BFNP)


def make_inputs(inputs, lay):
    """inputs: dict of full np arrays. lay: build output.
    Returns (meta, in_maps list of 8 dicts)."""
    Q = lay["Q"]
    nblk = lay["nblk"]

    node = np.asarray(inputs["node"], np.float32)
    wb = to_bf16(inputs["W_bases"])                       # [D, BF]
    wc = to_bf16(inputs["W_comb"])                        # [D, HBA]
    bcomb = np.tile(np.asarray(inputs["b_comb"], np.float32)[None, :], (P, 1))
    cbias = np.tile(np.asarray(inputs["conv_bias"], np.float32)[None, :], (P, 1))
    alphar = np.tile(np.asarray(inputs["gn_mean_scale"], np.float32)[None, :], (G, 1))
    gammar = np.tile(np.asarray(inputs["gn_weight"], np.float32)[None, :], (G, 1))
    br = np.tile(np.asarray(inputs["gn_bias"], np.float32)[None, :], (G, 1))

    meta = dict(Q=Q, nblk=nblk,
                Tr0=[int(x) for x in lay["Tr"][0]],
                Tr1=[int(x) for x in lay["Tr"][1]],
                Sr0=[int(x) for x in lay["Sr"][0]],
                Sr1=[int(x) for x in lay["Sr"][1]])

    in_maps = []
    for c in lay["cores"]:
        ntl = np.zeros((D, Q), BFNP)
        ntl[:, :c["nloc"]] = to_bf16(node[c["gperm"]]).T
        blob_bf = np.concatenate([
            ntl.ravel(), wb.ravel(), wc.ravel(),
            c["dstl_t"].ravel(), c["symw_t"].ravel()])
        blob_i16 = np.concatenate([c["i16_0"].ravel(), c["i16_1"].ravel()])
        blob_f32 = np.concatenate([
            bcomb.ravel(), c["gid_t"].ravel(), c["invcnt"].ravel(),
            alphar.ravel(), gammar.ravel(), br.ravel(), cbias.ravel(),
            c["nmask_t"].ravel()])
        in_maps.append(dict(blob_bf=blob_bf, blob_i16=blob_i16,
                            blob_f32=blob_f32))
    return meta, in_maps

# ============ device program ============

FP32 = mybir.dt.float32
BF16 = mybir.dt.bfloat16
I32 = mybir.dt.int32
I16 = mybir.dt.int16
AX = mybir.AxisListType
OP = mybir.AluOpType
ACTF = mybir.ActivationFunctionType
HBA = H * B * A   # 96
K = B * A         # 12


def build_program(nc, meta):
    Q = meta["Q"]
    nblk = meta["nblk"]
    Tr0, Tr1 = list(meta["Tr0"]), list(meta["Tr1"])
    Sr0, Sr1 = list(meta["Sr0"]), list(meta["Sr1"])
    sumT = sum(Tr0) + sum(Tr1)
    L0 = sum(8 * (t + s) for t, s in zip(Tr0, Sr0))   # cols of [16, L0]
    L1 = sum(8 * (t + s) for t, s in zip(Tr1, Sr1))
    TTmax = max(t0 + t1 for t0, t1 in zip(Tr0, Tr1))
    SPL = 4 * Q

    # ---- external blobs ---------------------------------------------------
    LBF = D * Q + D * BF + D * HBA + 2 * P * sumT
    LF32 = P * HBA + P * nblk + G + 3 * G * D + P * D + P * nblk
    blob_bf = nc.dram_tensor("blob_bf", [LBF], BF16, kind="ExternalInput")
    blob_i16 = nc.dram_tensor("blob_i16", [16 * (L0 + L1)], I16,
                              kind="ExternalInput")
    blob_f32 = nc.dram_tensor("blob_f32", [LF32], FP32, kind="ExternalInput")
    hout = nc.dram_tensor("hout", [Q, D], BF16, kind="ExternalOutput")

    o_ntl = 0
    o_wb = o_ntl + D * Q
    o_wc = o_wb + D * BF
    o_dstl = o_wc + D * HBA
    o_symw = o_dstl + P * sumT
    f_bcomb = 0
    f_gid = f_bcomb + P * HBA
    f_invc = f_gid + P * nblk
    f_alphar = f_invc + G
    f_gammar = f_alphar + G * D
    f_br = f_gammar + G * D
    f_cbias = f_br + G * D
    f_nmask = f_cbias + P * D

    with ExitStack() as ctx:
        tc = ctx.enter_context(tile.TileContext(nc))
        dram = ctx.enter_context(tc.tile_pool(name="dram", bufs=1, space="DRAM"))
        res = ctx.enter_context(tc.tile_pool(name="res", bufs=1))
        pa = ctx.enter_context(tc.tile_pool(name="pa", bufs=3))
        pgath = ctx.enter_context(tc.tile_pool(name="pgath", bufs=2))
        pidx = ctx.enter_context(tc.tile_pool(name="pidx", bufs=2))
        poh = ctx.enter_context(tc.tile_pool(name="poh", bufs=2))
        ptmp = ctx.enter_context(tc.tile_pool(name="ptmp", bufs=2))
        psm = ctx.enter_context(tc.tile_pool(name="psm", bufs=2))
        pd = ctx.enter_context(tc.tile_pool(name="pd", bufs=1))

        # ---- constants / resident tiles ----------------------------------
        wb_sb = res.tile([P, 2, BF], BF16)
        nc.sync.dma_start(wb_sb[:], blob_bf.ap()[o_wb:o_wb + D * BF]
                          .rearrange("(a p f) -> p a f", p=P, f=BF))
        wc_sb = res.tile([P, 2, HBA], BF16)
        nc.sync.dma_start(wc_sb[:], blob_bf.ap()[o_wc:o_wc + D * HBA]
                          .rearrange("(a p f) -> p a f", p=P, f=HBA))
        dstl_sb = res.tile([P, sumT], BF16)
        nc.sync.dma_start(dstl_sb[:], blob_bf.ap()[o_dstl:o_dstl + P * sumT]
                          .rearrange("(p t) -> p t", p=P))
        symw_bf = res.tile([P, sumT], BF16)
        nc.sync.dma_start(symw_bf[:], blob_bf.ap()[o_symw:o_symw + P * sumT]
                          .rearrange("(p t) -> p t", p=P))
        symw_sb = res.tile([P, sumT], FP32)   # scalar-engine scale must be FP32
        nc.vector.tensor_copy(symw_sb[:], symw_bf[:])
        bcomb_sb = res.tile([P, HBA], FP32)
        nc.sync.dma_start(bcomb_sb[:], blob_f32.ap()[f_bcomb:f_bcomb + P * HBA]
                          .rearrange("(p t) -> p t", p=P))
        gid_sb = res.tile([P, nblk], FP32)
        nc.sync.dma_start(gid_sb[:], blob_f32.ap()[f_gid:f_gid + P * nblk]
                          .rearrange("(p t) -> p t", p=P))
        invc_sb = res.tile([G, 1], FP32)
        nc.sync.dma_start(invc_sb[:], blob_f32.ap()[f_invc:f_invc + G]
                          .rearrange("(p t) -> p t", p=G))
        alphar_sb = res.tile([G, D], FP32)
        nc.sync.dma_start(alphar_sb[:], blob_f32.ap()[f_alphar:f_alphar + G * D]
                          .rearrange("(p t) -> p t", p=G))
        gammar_sb = res.tile([G, D], FP32)
        nc.sync.dma_start(gammar_sb[:], blob_f32.ap()[f_gammar:f_gammar + G * D]
                          .rearrange("(p t) -> p t", p=G))
        br_sb = res.tile([G, D], FP32)
        nc.sync.dma_start(br_sb[:], blob_f32.ap()[f_br:f_br + G * D]
                          .rearrange("(p t) -> p t", p=G))
        cbias_sb = res.tile([P, D], FP32)
        nc.sync.dma_start(cbias_sb[:], blob_f32.ap()[f_cbias:f_cbias + P * D]
                          .rearrange("(p t) -> p t", p=P))
        nmask_sb = res.tile([P, nblk], FP32)
        nc.sync.dma_start(nmask_sb[:], blob_f32.ap()[f_nmask:f_nmask + P * nblk]
                          .rearrange("(p t) -> p t", p=P))

        ident = res.tile([P, P], FP32)
        make_identity(nc, ident[:])
        ident_bf = res.tile([P, P], BF16)
        nc.vector.tensor_copy(ident_bf[:], ident[:])
        iota_i = res.tile([P, P], I32)
        nc.gpsimd.iota(iota_i[:], pattern=[[1, P]], base=0, channel_multiplier=0)
        iota_f = res.tile([P, P], FP32)
        nc.vector.tensor_copy(iota_f[:], iota_i[:])
        iota_bf = res.tile([P, P], BF16)
        nc.vector.tensor_copy(iota_bf[:], iota_i[:])
        iota_exp = res.tile([P, P, TTmax], BF16)
        nc.scalar.copy(iota_exp[:],
                       iota_bf[:].unsqueeze(2).broadcast_to([P, P, TTmax]))

        comb_sb = res.tile([P, nblk, HBA], BF16)
        goh_all = res.tile([P, nblk, G], BF16)
        hb_all = res.tile([P, nblk, D], BF16)

        # ---- 16->128 replication of gather-index streams ------------------
        rep0 = dram.tile([P, L0], I16)
        rep1 = dram.tile([P, L1], I16)
        for kk in range(8):
            nc.sync.dma_start(rep0[16 * kk:16 * (kk + 1), :],
                              blob_i16.ap()[0:16 * L0]
                              .rearrange("(a l) -> a l", a=16))
            nc.sync.dma_start(rep1[16 * kk:16 * (kk + 1), :],
                              blob_i16.ap()[16 * L0:16 * (L0 + L1)]
                              .rearrange("(a l) -> a l", a=16))

        # ---- stage A: local bases segment + comb, then AllGather ----------
        mybases = dram.tile([Q, BF], BF16)
        bases_all = dram.tile([NCORES * Q, BF], BF16)
        pab_cm = tc.tile_pool(name="pab", bufs=4, space="PSUM")
        pab = pab_cm.__enter__()
        pcb_cm = tc.tile_pool(name="pcb", bufs=2, space="PSUM")
        pcb = pcb_cm.__enter__()

        ntl_ap = blob_bf.ap()[o_ntl:o_ntl + D * Q].rearrange(
            "(a p n) -> p a n", p=P, n=Q)
        for b in range(nblk):
            lt2 = pa.tile([P, 2, P], BF16, tag="lt")
            nc.sync.dma_start(lt2[:], ntl_ap[:, :, b * P:(b + 1) * P])
            ps = pab.tile([P, BF], FP32, tag="ab")
            nc.tensor.matmul(ps[:], lt2[:, 0, :], wb_sb[:, 0, :],
                             start=True, stop=False)
            nc.tensor.matmul(ps[:], lt2[:, 1, :], wb_sb[:, 1, :],
                             start=False, stop=True)
            ob = pa.tile([P, BF], BF16, tag="ob")
            nc.vector.tensor_scalar(ob[:], ps[:], nmask_sb[:, b:b + 1], None,
                                    op0=OP.add)
            nc.sync.dma_start(mybases[b * P:(b + 1) * P, :], ob[:])
            cps = pcb.tile([P, HBA], FP32, tag="cps")
            nc.tensor.matmul(cps[:], lt2[:, 0, :], wc_sb[:, 0, :],
                             start=True, stop=False)
            nc.tensor.matmul(cps[:], lt2[:, 1, :], wc_sb[:, 1, :],
                             start=False, stop=True)
            nc.vector.tensor_tensor(comb_sb[:, b, :], cps[:], bcomb_sb[:],
                                    op=OP.add)

        pcb_cm.__exit__(None, None, None)
        pab_cm.__exit__(None, None, None)

        nc.gpsimd.collective_compute(
            "AllGather", OP.bypass,
            replica_groups=[list(range(NCORES))],
            ins=[mybases[:].opt()],
            outs=[bases_all[:].opt()])

        # ---- stage C: gather + aggregate + einsum + stats -----------------
        pacc_cm = tc.tile_pool(name="pacc", bufs=1, space="PSUM")
        pacc = pacc_cm.__enter__()
        pagg_cm = tc.tile_pool(name="pagg", bufs=2, space="PSUM")
        pagg = pagg_cm.__enter__()
        gsum_ps = pacc.tile([G, D], FP32)
        gsq_ps = pacc.tile([G, D], FP32)

        CH = 64                       # <=8192 idxs per dma_gather call
        c0 = 0
        c1 = 0
        tb = 0
        for b in range(nblk):
            T0, T1 = Tr0[b], Tr1[b]
            S0, S1 = Sr0[b], Sr1[b]
            W0, W1 = T0 + S0, T1 + S1
            TT = T0 + T1
            # gw[:, 0, :, :] = gathered messages; gw[:, 1, tile cols, :] =
            # symw-weighted messages (slot cols of plane 1 unused)
            gw = pgath.tile([P, 2, W0 + W1, BF], BF16, tag="gath")
            if b < 2:
                nc.gpsimd.memset(gw[:], 0.0)
            ix0 = pidx.tile([P, 8 * W0], I16, tag="ix0")
            nc.sync.dma_start(ix0[:], rep0[:, c0:c0 + 8 * W0])
            ix1 = pidx.tile([P, 8 * W1], I16, tag="ix1")
            nc.sync.dma_start(ix1[:], rep1[:, c1:c1 + 8 * W1])
            for w0 in range(0, W0, CH):
                w = min(CH, W0 - w0)
                nc.gpsimd.dma_gather(
                    out_ap=gw[:, 0, w0:w0 + w, :], in_ap=bases_all[0:SPL, :],
                    idxs_ap=ix0[:, 8 * w0:8 * (w0 + w)],
                    num_idxs=P * w, num_idxs_reg=P * w, elem_size=BF,
                    single_packet=False)
            for w1 in range(0, W1, CH):
                w = min(CH, W1 - w1)
                nc.gpsimd.dma_gather(
                    out_ap=gw[:, 0, W0 + w1:W0 + w1 + w, :],
                    in_ap=bases_all[SPL:NCORES * Q, :],
                    idxs_ap=ix1[:, 8 * w1:8 * (w1 + w)],
                    num_idxs=P * w, num_idxs_reg=P * w, elem_size=BF,
                    single_packet=False)
            c0 += 8 * W0
            c1 += 8 * W1

            # weighted copies of the tile columns (slot cols skipped)
            nc.vector.tensor_tensor(
                gw[:, 1, 0:T0, :], gw[:, 0, 0:T0, :],
                symw_sb[:, tb:tb + T0].unsqueeze(2).broadcast_to([P, T0, BF]),
                op=OP.mult)
            nc.vector.tensor_tensor(
                gw[:, 1, W0:W0 + T1, :], gw[:, 0, W0:W0 + T1, :],
                symw_sb[:, tb + T0:tb + TT].unsqueeze(2)
                .broadcast_to([P, T1, BF]),
                op=OP.mult)

            # block-level one-hot builds: oh[p_edge, x, t]
            oh = poh.tile([P, P, TTmax], BF16, tag="oh")
            nc.vector.tensor_tensor(
                oh[:, :, :TT],
                dstl_sb[:, tb:tb + TT].unsqueeze(1).broadcast_to([P, P, TT]),
                iota_exp[:, :, :TT], op=OP.is_equal)

            # one matmul per edge tile: moving [P, 2, BF] = (msg | w*msg)
            ps_c = pagg.tile([P, 2, BF], FP32, tag="aggc")
            for t in range(TT):
                mcol = t if t < T0 else W0 + (t - T0)
                nc.tensor.matmul(ps_c[:], oh[:, :, t], gw[:, :, mcol, :],
                                 start=(t == 0), stop=(t == TT - 1))
            # max: halve (overlap-safe) then one strided reduce over both
            # ranges into aggT[:, :, 8:12]
            m0, m1 = (S0 + 1) // 2, (S1 + 1) // 2
            hmax = ptmp.tile([P, m0 + m1, BF], BF16, tag="hmax")
            nc.vector.tensor_tensor(hmax[:, :m0, :],
                                    gw[:, 0, T0:T0 + m0, :],
                                    gw[:, 0, T0 + S0 - m0:T0 + S0, :],
                                    op=OP.max)
            nc.vector.tensor_tensor(hmax[:, m0:m0 + m1, :],
                                    gw[:, 0, W0 + T1:W0 + T1 + m1, :],
                                    gw[:, 0, W0 + W1 - m1:W0 + W1, :],
                                    op=OP.max)
            aggT = psm.tile([P, F, K], BF16, tag="aggT")
            nc.vector.tensor_reduce(
                aggT[:, :, 2 * B:3 * B].transpose([0, 2, 1]),
                hmax[:].rearrange("p s (bb f) -> p (bb f) s", bb=B),
                axis=AX.X, op=OP.max, opt_input=False)
            # sym (a=0) / sum (a=1) from psum, transposed to [P, F, b]
            nc.scalar.copy(aggT[:, :, 0:B].transpose([0, 2, 1]),
                           ps_c[:, 1, :].rearrange("p (bb f) -> p bb f", bb=B))
            nc.scalar.copy(aggT[:, :, B:2 * B].transpose([0, 2, 1]),
                           ps_c[:, 0, :].rearrange("p (bb f) -> p bb f", bb=B))

            # einsum: tmp[p,h,f,k] = aggT[p,f,k] * comb[p,h,k]; tree-reduce k
            tmp = ptmp.tile([P, H, F, K], BF16, tag="tmp")
            nc.vector.tensor_tensor(
                tmp[:],
                aggT[:].unsqueeze(1).broadcast_to([P, H, F, K]),
                comb_sb[:, b, :].rearrange("p (h k) -> p h k", h=H)
                .unsqueeze(2).broadcast_to([P, H, F, K]),
                op=OP.mult)
            t6 = ptmp.tile([P, H, F, 6], BF16, tag="t6")
            nc.vector.tensor_tensor(t6[:], tmp[:, :, :, 0:6],
                                    tmp[:, :, :, 6:12], op=OP.add)
            t3 = ptmp.tile([P, H, F, 3], BF16, tag="t3")
            nc.vector.tensor_tensor(t3[:], t6[:, :, :, 0:3],
                                    t6[:, :, :, 3:6], op=OP.add)
            hbt = psm.tile([P, D], FP32, tag="hbt")
            nc.vector.tensor_reduce(hbt[:], t3[:], axis=AX.X, op=OP.add,
                                    opt_input=False)
            nc.vector.tensor_tensor(hb_all[:, b, :], hbt[:], cbias_sb[:],
                                    op=OP.add)
            hsq = psm.tile([P, D], BF16, tag="hsq")
            nc.scalar.square(hsq[:], hb_all[:, b, :])

            # graph one-hot + stats
            goh = goh_all[:, b, :]
            nc.vector.tensor_scalar(goh, iota_f[:, :G],
                                    gid_sb[:, b:b + 1], None, op0=OP.is_equal)
            nc.tensor.matmul(gsum_ps[:], goh, hb_all[:, b, :],
                             start=(b == 0), stop=(b == nblk - 1))
            nc.tensor.matmul(gsq_ps[:], goh, hsq[:],
                             start=(b == 0), stop=(b == nblk - 1))
            tb += TT

        # ---- stage D: per-graph stats ------------------------------------
        stats = res.tile([G, 2, D], FP32)    # q' | rstd*gamma
        mean = pd.tile([G, D], FP32, tag="mean")
        nc.vector.tensor_scalar(mean[:], gsum_ps[:], invc_sb[:, 0:1], None,
                                op0=OP.mult)
        ex2 = pd.tile([G, D], FP32, tag="ex2")
        nc.vector.tensor_scalar(ex2[:], gsq_ps[:], invc_sb[:, 0:1], None,
                                op0=OP.mult)
        meansc = pd.tile([G, D], FP32, tag="meansc")
        nc.vector.tensor_tensor(meansc[:], mean[:], alphar_sb[:], op=OP.mult)
        t2 = pd.tile([G, D], FP32, tag="t2")
        nc.vector.scalar_tensor_tensor(t2[:], mean[:], 2.0, meansc[:],
                                       op0=OP.mult, op1=OP.subtract)
        var = pd.tile([G, D], FP32, tag="var")
        nc.vector.tensor_tensor(var[:], meansc[:], t2[:], op=OP.mult)
        nc.vector.tensor_tensor(var[:], ex2[:], var[:], op=OP.subtract)
        nc.vector.tensor_scalar(var[:], var[:], EPS, None, op0=OP.add)
        sd = pd.tile([G, D], FP32, tag="sd")
        nc.scalar.activation(sd[:], var[:], ACTF.Sqrt)
        rstd = pd.tile([G, D], FP32, tag="rstd")
        nc.vector.reciprocal(rstd[:], sd[:])
        nc.vector.tensor_tensor(stats[:, 1, :], rstd[:], gammar_sb[:],
                                op=OP.mult)
        # q = meansc * (rstd*gamma) - beta  -> out = h*s - q
        nc.vector.tensor_tensor(stats[:, 0, :], meansc[:],
                                stats[:, 1, :], op=OP.mult)
        nc.vector.tensor_tensor(stats[:, 0, :], stats[:, 0, :],
                                br_sb[:], op=OP.subtract)

        # ---- stage E: normalize + relu + out -----------------------------
        pagg_cm.__exit__(None, None, None)
        pacc_cm.__exit__(None, None, None)
        pe = ctx.enter_context(tc.tile_pool(name="pe", bufs=2, space="PSUM"))
        stats_bf_t = res.tile([G, 2, D], BF16)
        nc.vector.tensor_copy(stats_bf_t[:], stats[:])
        for b in range(nblk):
            gt_ps = pe.tile([G, P], BF16, tag="gt")
            nc.tensor.transpose(gt_ps[:], goh_all[:, b, :], ident_bf[:])
            gt = psm.tile([G, P], BF16, tag="gts")
            nc.scalar.copy(gt[:], gt_ps[:])
            stats_bf = stats_bf_t[:]
            bc = pe.tile([P, 2, D], FP32, tag="bc")
            nc.tensor.matmul(bc[:], gt[:], stats_bf, start=True, stop=True)
            hc = psm.tile([P, D], FP32, tag="hc")
            nc.vector.tensor_tensor(hc[:], hb_all[:, b, :], bc[:, 1, :],
                                    op=OP.mult)
            nc.vector.tensor_tensor(hc[:], hc[:], bc[:, 0, :], op=OP.subtract)
            ho = psm.tile([P, D], BF16, tag="ho")
            nc.scalar.activation(ho[:], hc[:], ACTF.Relu)
            nc.sync.dma_start(hout.ap()[b * P:(b + 1) * P, :], ho[:])

    return nc

# ======================= entry point =======================


def kernel(**inputs) -> np.ndarray:
    inputs = {k: np.asarray(v) for k, v in inputs.items()}
    lay = build(inputs["edge_index"].astype(np.int64),
                inputs["batch"].astype(np.int64))
    meta, in_maps = make_inputs(inputs, lay)

    nc = bacc.Bacc("TRN2", target_bir_lowering=False, debug=False,
                   num_devices=NCORES)
    build_program(nc, meta)
    nc.compile()
    res = bass_utils.run_bass_kernel_spmd(nc, in_maps,
                                          core_ids=list(range(NCORES)))
    outs = [res.results[c]["hout"] for c in range(NCORES)]
    kernel.last = dict(nc=nc, in_maps=in_maps, lay=lay, meta=meta)
    return unshard(lay, outs)


# revision 22
# speedup vs baseline: 5.7348x; 1.0753x over previous
"""EGConv layer (gnn_message_passing) on 8 Trainium2 NeuronCores.

Self-contained: kernel(**inputs) -> np.ndarray [50000, 256] float32.

Strategy: graph-aligned 1D node partition over 8 cores (GraphNorm fully
core-local), per-core degree-sorted node permutation, dst-sorted edge
streams. Each core computes the bases rows of only ITS nodes (plus comb
weights) from its local node shard; a device AllGather assembles the
full [8Q, BF] bf16 bases table on every core. Messages are fetched by
dma_gather from two int16-indexable halves of that table (split at the
core-aligned row 4Q); sum/sym aggregation via block-level one-hot
matmuls on the tensor engine; max via slot-layout gather + halve +
strided max-reduce; per-node einsum in bf16 on the vector engine;
GraphNorm via per-graph one-hot matmuls. Host->device traffic is
minimized: inputs packed into three dtype-blobs (bf16/int16/fp32),
gather-index streams shipped 16-row (replicated to 128 on device),
output returned in bf16. The SPMD program is identical across cores;
all per-core variation is in the data.
"""
import sys
for _p in ("/opt/trn_rl_repo", "/root/.axon_site/_ro/trn_rl_repo"):
    if _p not in sys.path:
        sys.path.insert(0, _p)

import os
import numpy as np
import ml_dtypes
from contextlib import ExitStack

import concourse.bass as bass
import concourse.mybir as mybir
import concourse.tile as tile
from concourse import bacc, bass_utils

BFNP = ml_dtypes.bfloat16

# ======================= host-side graph preprocessing =======================

N, E, D = 50000, 800000, 256
H, B, A = 8, 4, 3
F = D // H          # 32
BF = B * F          # 128
G = 64
EPS = 1e-5
NCORES = 8
P = 128
NEG = -1e30


def build(edge_index: np.ndarray, batch: np.ndarray):
    """edge_index [2,E] int32, batch [N] int32 sorted. Returns layout dict."""
    src_all = np.concatenate([edge_index[0], np.arange(N, dtype=np.int64)])
    dst_all = np.concatenate([edge_index[1], np.arange(N, dtype=np.int64)])

    deg = np.bincount(dst_all, minlength=N).astype(np.float64)
    dinv = np.where(deg > 0, 1.0 / np.sqrt(deg), 0.0).astype(np.float32)
    symw_all = (dinv[src_all] * dinv[dst_all]).astype(np.float32)

    # graph-aligned 8-way shard
    gcnt = np.bincount(batch, minlength=G)
    gend = np.cumsum(gcnt)            # node index where graph g ends
    cuts = [0]
    for c in range(1, NCORES):
        target = round(N * c / NCORES)
        gi = np.argmin(np.abs(gend - target))
        cuts.append(int(gend[gi]))
    cuts.append(N)
    cuts = sorted(set(cuts))
    assert len(cuts) == NCORES + 1, cuts
    cuts_a = np.asarray(cuts, dtype=np.int64)
    src_core = np.searchsorted(cuts_a, src_all, side="right") - 1

    # pass 1: per-core degree-sorted permutation
    cores = []
    for c in range(NCORES):
        n0, n1 = cuts[c], cuts[c + 1]
        nloc = n1 - n0
        local_deg = deg[n0:n1]
        # secondary key: T0-range in-degree (srcs on cores 0-3), to tighten
        # per-range slot rectangles
        ldeg0 = np.bincount(dst_all[(dst_all >= n0) & (dst_all < n1)
                                    & (src_core < 4)] - n0,
                            minlength=nloc).astype(np.float64)
        perm = np.lexsort((-ldeg0, -local_deg)).astype(np.int64)  # desc
        gperm = perm + n0                      # new local id -> global id
        inv = np.empty(nloc, dtype=np.int64)
        inv[perm] = np.arange(nloc)            # orig local -> new local id
        cores.append(dict(n0=n0, n1=n1, nloc=nloc, gperm=gperm, inv=inv))

    maxloc = max(c["nloc"] for c in cores)
    Q = (maxloc // P + 1) * P                  # strictly > every nloc
    nblk = Q // P
    SPL = 4 * Q                                # T0/T1 split row (core-aligned)
    assert SPL - 1 <= 32767 and 4 * Q - 1 <= 32767

    ginv = np.empty(N, dtype=np.int64)         # orig global -> permuted row
    for c, core in enumerate(cores):
        ginv[core["gperm"]] = c * Q + np.arange(core["nloc"])

    # pass 2: per-core edge streams + global Tr/Sr
    nR = 2
    for core in cores:
        n0, n1 = core["n0"], core["n1"]
        emask = (dst_all >= n0) & (dst_all < n1)
        esrc = src_all[emask]
        edstl = core["inv"][dst_all[emask] - n0]   # new local dst id
        esym = symw_all[emask]
        order = np.argsort(edstl, kind="stable")
        core["esrc"], core["edstl"], core["esym"] = \
            esrc[order], edstl[order], esym[order]
        core["erow"] = ginv[core["esrc"]]          # permuted source row

    Tr = np.zeros((nR, nblk), dtype=np.int64)
    Sr = np.zeros((nR, nblk), dtype=np.int64)
    for c in cores:
        blk = c["edstl"] // P
        rng = (c["erow"] >= SPL).astype(np.int64)
        for r in range(nR):
            cnt = np.bincount(blk[rng == r], minlength=nblk)
            Tr[r] = np.maximum(Tr[r], (cnt + P - 1) // P)
            dl = c["edstl"][rng == r]
            nd = np.bincount(dl, minlength=nblk * P).reshape(nblk, P)
            Sr[r] = np.maximum(Sr[r], nd.max(axis=1))
    Tr = np.maximum(Tr, 1)
    Sr = np.maximum(Sr, 1)

    PAD0, PAD1 = SPL - 1, 4 * Q - 1   # NEG tail rows (cores 3 / 7), per-range

    sumTT = int((Tr[0] + Tr[1]).sum())
    for c in cores:
        nloc = c["nloc"]
        dstl_t = np.full((P, sumTT), -1.0, dtype=BFNP)
        symw_t = np.zeros((P, sumTT), dtype=BFNP)
        flat_r = [[], []]
        blk = c["edstl"] // P
        rng = (c["erow"] >= SPL).astype(np.int64)
        tcol = 0
        for b in range(nblk):
            for r in range(nR):
                m = (blk == b) & (rng == r)
                srow = c["erow"][m] - (SPL if r else 0)
                dl = c["edstl"][m] - b * P
                sw = c["esym"][m]
                k = len(srow)
                T, S = int(Tr[r][b]), int(Sr[r][b])
                pad = PAD1 if r else PAD0
                ef = np.full(P * T, pad, dtype=np.int64)
                ef[:k] = srow
                flat_r[r].append(ef)
                cols = tcol + np.arange(k) // P
                rows = np.arange(k) % P
                dstl_t[rows, cols] = dl.astype(np.float32)
                symw_t[rows, cols] = sw
                tcol += T
                sf = np.full(P * S, pad, dtype=np.int64)
                if k:
                    marks = np.flatnonzero(np.diff(dl, prepend=-1))
                    slot = np.arange(k) - np.repeat(marks, np.diff(
                        np.append(marks, k)))
                    sf[slot * P + dl] = srow
                    # pad slots of nodes that HAVE >=1 edge in this range:
                    # duplicate the node's first edge (max unchanged, avoids
                    # a NEG-row fetch)
                    first = np.full(P, -1, dtype=np.int64)
                    first[dl[marks]] = srow[marks]
                    degr = np.zeros(P, dtype=np.int64)
                    dcnt = np.diff(np.append(marks, k))
                    degr[dl[marks]] = dcnt
                    for s in range(S):
                        lane = np.flatnonzero((degr > 0) & (degr <= s))
                        sf[s * P + lane] = first[lane]
                flat_r[r].append(sf)
        i16 = []
        for r in range(nR):
            fl = np.concatenate(flat_r[r])
            L = len(fl)
            w = np.zeros((16, L // 16), dtype=np.int16)
            w[np.arange(L) % 16, np.arange(L) // 16] = fl
            i16.append(w)

        gl0 = batch[c["n0"]]
        ngid = np.full(nblk * P, -1.0, dtype=np.float32)
        ngid[:nloc] = (batch[c["gperm"]] - gl0).astype(np.float32)
        gid_t = ngid.reshape(nblk, P).T.copy()

        # stage-E stats-gather index stream (graph id per node, block-major)
        gfl = np.where(ngid < 0, 0, ngid).astype(np.int64)
        L2f = nblk * P
        w2 = np.zeros((16, L2f // 16), dtype=np.int16)
        w2[np.arange(L2f) % 16, np.arange(L2f) // 16] = gfl
        gidx16 = w2

        nmv = np.zeros(nblk * P, dtype=np.float32)
        nmv[nloc:] = NEG
        nmask_t = nmv.reshape(nblk, P).T.copy()

        icnt = np.ones((G, 1), dtype=np.float32)
        glo = np.bincount((batch[c["n0"]:c["n1"]] - gl0), minlength=G)
        icnt[glo > 0, 0] = (1.0 / glo[glo > 0]).astype(np.float32)
        invc = np.ones((G, 1), dtype=np.float32)
        invc[:icnt.shape[0]] = icnt

        c.update(dstl_t=dstl_t, symw_t=symw_t, i16_0=i16[0], i16_1=i16[1],
                 gidx16=gidx16, gid_t=gid_t, nmask_t=nmask_t, invcnt=invc)

    return dict(cores=cores, nblk=nblk, Q=Q, Tr=Tr, Sr=Sr, cuts=cuts)


def unshard(layout, per_core_out):
    full = np.zeros((N, D), dtype=np.float32)
    for c, out in zip(layout["cores"], per_core_out):
        full[c["gperm"]] = out[:c["nloc"]].astype(np.float32)
    return full

# ============ input-map construction ============


def to_bf16(x):
    return np.asarray(x, np.float32).astype(BFNP)


def make_inputs(inputs, lay):
    """inputs: dict of full np arrays. lay: build output.
    Returns (meta, in_maps list of 8 dicts)."""
    Q = lay["Q"]
    nblk = lay["nblk"]

    node = np.asarray(inputs["node"], np.float32)
    wb = to_bf16(inputs["W_bases"])                       # [D, BF]
    wc = to_bf16(inputs["W_comb"])                        # [D, HBA]
    bcomb = np.tile(np.asarray(inputs["b_comb"], np.float32)[None, :], (P, 1))
    cbias = np.tile(np.asarray(inputs["conv_bias"], np.float32)[None, :], (P, 1))
    alphar = np.tile(np.asarray(inputs["gn_mean_scale"], np.float32)[None, :], (G, 1))
    gammar = np.tile(np.asarray(inputs["gn_weight"], np.float32)[None, :], (G, 1))
    br = np.tile(np.asarray(inputs["gn_bias"], np.float32)[None, :], (G, 1))

    meta = dict(Q=Q, nblk=nblk,
                Tr0=[int(x) for x in lay["Tr"][0]],
                Tr1=[int(x) for x in lay["Tr"][1]],
                Sr0=[int(x) for x in lay["Sr"][0]],
                Sr1=[int(x) for x in lay["Sr"][1]])

    in_maps = []
    for c in lay["cores"]:
        ntl = np.zeros((D, Q), BFNP)
        ntl[:, :c["nloc"]] = to_bf16(node[c["gperm"]]).T
        blob_bf = np.concatenate([
            ntl.ravel(), wb.ravel(), wc.ravel(),
            c["dstl_t"].ravel(), c["symw_t"].ravel()])
        blob_i16 = np.hstack([c["i16_0"], c["i16_1"], c["gidx16"]]).ravel()
        blob_f32 = np.concatenate([
            bcomb.ravel(), c["gid_t"].ravel(), c["invcnt"].ravel(),
            alphar.ravel(), gammar.ravel(), br.ravel(), cbias.ravel(),
            c["nmask_t"].ravel()])
        in_maps.append(dict(blob_bf=blob_bf, blob_i16=blob_i16,
                            blob_f32=blob_f32))
    return meta, in_maps

# ============ device program ============

FP32 = mybir.dt.float32
BF16 = mybir.dt.bfloat16
I32 = mybir.dt.int32
I16 = mybir.dt.int16
AX = mybir.AxisListType
OP = mybir.AluOpType
ACTF = mybir.ActivationFunctionType
HBA = H * B * A   # 96
K = B * A         # 12


def build_program(nc, meta):
    Q = meta["Q"]
    nblk = meta["nblk"]
    Tr0, Tr1 = list(meta["Tr0"]), list(meta["Tr1"])
    Sr0, Sr1 = list(meta["Sr0"]), list(meta["Sr1"])
    sumT = sum(Tr0) + sum(Tr1)
    L0 = sum(8 * (t + s) for t, s in zip(Tr0, Sr0))   # cols of [16, L0]
    L1 = sum(8 * (t + s) for t, s in zip(Tr1, Sr1))
    L2 = 8 * nblk                                      # stage-E gid stream
    TTmax = max(t0 + t1 for t0, t1 in zip(Tr0, Tr1))
    SPL = 4 * Q

    # ---- external blobs ---------------------------------------------------
    LBF = D * Q + D * BF + D * HBA + 2 * P * sumT
    LF32 = P * HBA + P * nblk + G + 3 * G * D + P * D + P * nblk
    blob_bf = nc.dram_tensor("blob_bf", [LBF], BF16, kind="ExternalInput")
    blob_i16 = nc.dram_tensor("blob_i16", [16 * (L0 + L1 + L2)], I16,
                              kind="ExternalInput")
    blob_f32 = nc.dram_tensor("blob_f32", [LF32], FP32, kind="ExternalInput")
    hout = nc.dram_tensor("hout", [Q, D], BF16, kind="ExternalOutput")

    o_ntl = 0
    o_wb = o_ntl + D * Q
    o_wc = o_wb + D * BF
    o_dstl = o_wc + D * HBA
    o_symw = o_dstl + P * sumT
    f_bcomb = 0
    f_gid = f_bcomb + P * HBA
    f_invc = f_gid + P * nblk
    f_alphar = f_invc + G
    f_gammar = f_alphar + G * D
    f_br = f_gammar + G * D
    f_cbias = f_br + G * D
    f_nmask = f_cbias + P * D

    with ExitStack() as ctx:
        tc = ctx.enter_context(tile.TileContext(nc))
        dram = ctx.enter_context(tc.tile_pool(name="dram", bufs=1, space="DRAM"))
        res = ctx.enter_context(tc.tile_pool(name="res", bufs=1))
        pa = ctx.enter_context(tc.tile_pool(name="pa", bufs=3))
        pgath = ctx.enter_context(tc.tile_pool(name="pgath", bufs=2))
        pidx = ctx.enter_context(tc.tile_pool(name="pidx", bufs=2))
        poh = ctx.enter_context(tc.tile_pool(name="poh", bufs=2))
        ptmp = ctx.enter_context(tc.tile_pool(name="ptmp", bufs=2))
        psm = ctx.enter_context(tc.tile_pool(name="psm", bufs=2))
        pd = ctx.enter_context(tc.tile_pool(name="pd", bufs=1))

        # ---- constants / resident tiles ----------------------------------
        wb_sb = res.tile([P, 2, BF], BF16)
        nc.sync.dma_start(wb_sb[:], blob_bf.ap()[o_wb:o_wb + D * BF]
                          .rearrange("(a p f) -> p a f", p=P, f=BF))
        wc_sb = res.tile([P, 2, HBA], BF16)
        nc.sync.dma_start(wc_sb[:], blob_bf.ap()[o_wc:o_wc + D * HBA]
                          .rearrange("(a p f) -> p a f", p=P, f=HBA))
        dstl_sb = res.tile([P, sumT], BF16)
        nc.sync.dma_start(dstl_sb[:], blob_bf.ap()[o_dstl:o_dstl + P * sumT]
                          .rearrange("(p t) -> p t", p=P))
        symw_bf = res.tile([P, sumT], BF16)
        nc.sync.dma_start(symw_bf[:], blob_bf.ap()[o_symw:o_symw + P * sumT]
                          .rearrange("(p t) -> p t", p=P))
        symw_sb = res.tile([P, sumT], FP32)   # scalar-engine scale must be FP32
        nc.vector.tensor_copy(symw_sb[:], symw_bf[:])
        bcomb_sb = res.tile([P, HBA], FP32)
        nc.sync.dma_start(bcomb_sb[:], blob_f32.ap()[f_bcomb:f_bcomb + P * HBA]
                          .rearrange("(p t) -> p t", p=P))
        gid_sb = res.tile([P, nblk], FP32)
        nc.sync.dma_start(gid_sb[:], blob_f32.ap()[f_gid:f_gid + P * nblk]
                          .rearrange("(p t) -> p t", p=P))
        invc_sb = res.tile([G, 1], FP32)
        nc.sync.dma_start(invc_sb[:], blob_f32.ap()[f_invc:f_invc + G]
                          .rearrange("(p t) -> p t", p=G))
        alphar_sb = res.tile([G, D], FP32)
        nc.sync.dma_start(alphar_sb[:], blob_f32.ap()[f_alphar:f_alphar + G * D]
                          .rearrange("(p t) -> p t", p=G))
        gammar_sb = res.tile([G, D], FP32)
        nc.sync.dma_start(gammar_sb[:], blob_f32.ap()[f_gammar:f_gammar + G * D]
                          .rearrange("(p t) -> p t", p=G))
        br_sb = res.tile([G, D], FP32)
        nc.sync.dma_start(br_sb[:], blob_f32.ap()[f_br:f_br + G * D]
                          .rearrange("(p t) -> p t", p=G))
        cbias_sb = res.tile([P, D], FP32)
        nc.sync.dma_start(cbias_sb[:], blob_f32.ap()[f_cbias:f_cbias + P * D]
                          .rearrange("(p t) -> p t", p=P))
        nmask_sb = res.tile([P, nblk], FP32)
        nc.sync.dma_start(nmask_sb[:], blob_f32.ap()[f_nmask:f_nmask + P * nblk]
                          .rearrange("(p t) -> p t", p=P))

        iota_i = res.tile([P, P], I32)
        nc.gpsimd.iota(iota_i[:], pattern=[[1, P]], base=0, channel_multiplier=0)
        iota_f = res.tile([P, P], FP32)
        nc.vector.tensor_copy(iota_f[:], iota_i[:])
        iota_bf = res.tile([P, P], BF16)
        nc.vector.tensor_copy(iota_bf[:], iota_i[:])
        iota_exp = res.tile([P, P, TTmax], BF16)
        nc.scalar.copy(iota_exp[:],
                       iota_bf[:].unsqueeze(2).broadcast_to([P, P, TTmax]))

        comb_sb = res.tile([P, nblk, HBA], BF16)
        hb_all = res.tile([P, nblk, D], BF16)

        # ---- 16->128 replication of gather-index streams ------------------
        # layout: [0,L0) range0, [L0,L0+L1) range1, [L0+L1,..) stage-E gids
        rep = dram.tile([P, L0 + L1 + L2], I16)
        for kk in range(8):
            nc.sync.dma_start(rep[16 * kk:16 * (kk + 1), :],
                              blob_i16.ap().rearrange("(a l) -> a l", a=16))

        # ---- stage A: local bases segment + comb, then AllGather ----------
        mybases = dram.tile([Q, BF], BF16)
        bases_all = dram.tile([NCORES * Q, BF], BF16)
        pab_cm = tc.tile_pool(name="pab", bufs=4, space="PSUM")
        pab = pab_cm.__enter__()
        pcb_cm = tc.tile_pool(name="pcb", bufs=2, space="PSUM")
        pcb = pcb_cm.__enter__()

        ntl_ap = blob_bf.ap()[o_ntl:o_ntl + D * Q].rearrange(
            "(a p n) -> p a n", p=P, n=Q)
        for b in range(nblk):
            lt2 = pa.tile([P, 2, P], BF16, tag="lt")
            nc.sync.dma_start(lt2[:], ntl_ap[:, :, b * P:(b + 1) * P])
            ps = pab.tile([P, BF], FP32, tag="ab")
            nc.tensor.matmul(ps[:], lt2[:, 0, :], wb_sb[:, 0, :],
                             start=True, stop=False)
            nc.tensor.matmul(ps[:], lt2[:, 1, :], wb_sb[:, 1, :],
                             start=False, stop=True)
            ob = pa.tile([P, BF], BF16, tag="ob")
            nc.vector.tensor_scalar(ob[:], ps[:], nmask_sb[:, b:b + 1], None,
                                    op0=OP.add)
            nc.sync.dma_start(mybases[b * P:(b + 1) * P, :], ob[:])
            cps = pcb.tile([P, HBA], FP32, tag="cps")
            nc.tensor.matmul(cps[:], lt2[:, 0, :], wc_sb[:, 0, :],
                             start=True, stop=False)
            nc.tensor.matmul(cps[:], lt2[:, 1, :], wc_sb[:, 1, :],
                             start=False, stop=True)
            nc.vector.tensor_tensor(comb_sb[:, b, :], cps[:], bcomb_sb[:],
                                    op=OP.add)

        pcb_cm.__exit__(None, None, None)
        pab_cm.__exit__(None, None, None)

        nc.gpsimd.collective_compute(
            "AllGather", OP.bypass,
            replica_groups=[list(range(NCORES))],
            ins=[mybases[:].opt()],
            outs=[bases_all[:].opt()])

        # ---- stage C: gather + aggregate + einsum + stats -----------------
        pacc_cm = tc.tile_pool(name="pacc", bufs=1, space="PSUM")
        pacc = pacc_cm.__enter__()
        pagg_cm = tc.tile_pool(name="pagg", bufs=2, space="PSUM")
        pagg = pagg_cm.__enter__()
        gsum_ps = pacc.tile([G, D], FP32)
        gsq_ps = pacc.tile([G, D], FP32)

        CH = 64                       # <=8192 idxs per dma_gather call
        c0 = 0
        c1 = 0
        tb = 0
        for b in range(nblk):
            T0, T1 = Tr0[b], Tr1[b]
            S0, S1 = Sr0[b], Sr1[b]
            W0, W1 = T0 + S0, T1 + S1
            TT = T0 + T1
            # gw[:, 1, :, :] = gathered messages; gw[:, 0, tile cols, :] =
            # symw-weighted messages (slot cols of plane 0 unused)
            gw = pgath.tile([P, 2, W0 + W1, BF], BF16, tag="gath")
            if b < 2:
                nc.gpsimd.memset(gw[:], 0.0)
            ix0 = pidx.tile([P, 8 * W0], I16, tag="ix0")
            nc.sync.dma_start(ix0[:], rep[:, c0:c0 + 8 * W0])
            ix1 = pidx.tile([P, 8 * W1], I16, tag="ix1")
            nc.sync.dma_start(ix1[:], rep[:, L0 + c1:L0 + c1 + 8 * W1])
            for w0 in range(0, W0, CH):
                w = min(CH, W0 - w0)
                nc.gpsimd.dma_gather(
                    out_ap=gw[:, 1, w0:w0 + w, :], in_ap=bases_all[0:SPL, :],
                    idxs_ap=ix0[:, 8 * w0:8 * (w0 + w)],
                    num_idxs=P * w, num_idxs_reg=P * w, elem_size=BF,
                    single_packet=False)
            for w1 in range(0, W1, CH):
                w = min(CH, W1 - w1)
                nc.gpsimd.dma_gather(
                    out_ap=gw[:, 1, W0 + w1:W0 + w1 + w, :],
                    in_ap=bases_all[SPL:NCORES * Q, :],
                    idxs_ap=ix1[:, 8 * w1:8 * (w1 + w)],
                    num_idxs=P * w, num_idxs_reg=P * w, elem_size=BF,
                    single_packet=False)
            c0 += 8 * W0
            c1 += 8 * W1

            # weighted copies of the tile columns (slot cols skipped)
            nc.vector.tensor_tensor(
                gw[:, 0, 0:T0, :], gw[:, 1, 0:T0, :],
                symw_sb[:, tb:tb + T0].unsqueeze(2).broadcast_to([P, T0, BF]),
                op=OP.mult)
            nc.vector.tensor_tensor(
                gw[:, 0, W0:W0 + T1, :], gw[:, 1, W0:W0 + T1, :],
                symw_sb[:, tb + T0:tb + TT].unsqueeze(2)
                .broadcast_to([P, T1, BF]),
                op=OP.mult)

            # block-level one-hot builds: oh[p_edge, x, t]
            oh = poh.tile([P, P, TTmax], BF16, tag="oh")
            nc.vector.tensor_tensor(
                oh[:, :, :TT],
                dstl_sb[:, tb:tb + TT].unsqueeze(1).broadcast_to([P, P, TT]),
                iota_exp[:, :, :TT], op=OP.is_equal)

            # one matmul per edge tile: moving [P, 2, BF] = (msg | w*msg)
            ps_c = pagg.tile([P, 2, BF], FP32, tag="aggc")
            for t in range(TT):
                mcol = t if t < T0 else W0 + (t - T0)
                nc.tensor.matmul(ps_c[:], oh[:, :, t], gw[:, :, mcol, :],
                                 start=(t == 0), stop=(t == TT - 1))
            # max: halve (overlap-safe) then one strided reduce over both
            # ranges into aggT[:, :, 8:12]
            m0, m1 = (S0 + 1) // 2, (S1 + 1) // 2
            hmax = ptmp.tile([P, m0 + m1, BF], BF16, tag="hmax")
            nc.vector.tensor_tensor(hmax[:, :m0, :],
                                    gw[:, 1, T0:T0 + m0, :],
                                    gw[:, 1, T0 + S0 - m0:T0 + S0, :],
                                    op=OP.max)
            nc.vector.tensor_tensor(hmax[:, m0:m0 + m1, :],
                                    gw[:, 1, W0 + T1:W0 + T1 + m1, :],
                                    gw[:, 1, W0 + W1 - m1:W0 + W1, :],
                                    op=OP.max)
            aggT = psm.tile([P, F, K], BF16, tag="aggT")
            nc.vector.tensor_reduce(
                aggT[:, :, 2 * B:3 * B].transpose([0, 2, 1]),
                hmax[:].rearrange("p s (bb f) -> p (bb f) s", bb=B),
                axis=AX.X, op=OP.max, opt_input=False)
            # sym (a=0) / sum (a=1) from psum, transposed to [P, F, b]
            nc.scalar.copy(
                aggT[:, :, 0:2 * B].rearrange("p f (a bb) -> p a bb f", a=2),
                ps_c[:].rearrange("p a (bb f) -> p a bb f", bb=B))

            # einsum: tmp[p,h,f,k] = aggT[p,f,k] * comb[p,h,k]; reduce k
            tmp = ptmp.tile([P, H, F, K], BF16, tag="tmp")
            nc.vector.tensor_tensor(
                tmp[:],
                aggT[:].unsqueeze(1).broadcast_to([P, H, F, K]),
                comb_sb[:, b, :].rearrange("p (h k) -> p h k", h=H)
                .unsqueeze(2).broadcast_to([P, H, F, K]),
                op=OP.mult)
            hbt = psm.tile([P, D], FP32, tag="hbt")
            nc.vector.tensor_reduce(hbt[:], tmp[:], axis=AX.X, op=OP.add,
                                    opt_input=False)
            nc.vector.tensor_tensor(hb_all[:, b, :], hbt[:], cbias_sb[:],
                                    op=OP.add)
            hsq = psm.tile([P, D], BF16, tag="hsq")
            nc.scalar.square(hsq[:], hb_all[:, b, :])

            # graph one-hot + stats
            goh = psm.tile([P, G], BF16, tag="goh")
            nc.vector.tensor_scalar(goh[:], iota_f[:, :G],
                                    gid_sb[:, b:b + 1], None, op0=OP.is_equal)
            nc.tensor.matmul(gsum_ps[:], goh[:], hb_all[:, b, :],
                             start=(b == 0), stop=(b == nblk - 1))
            nc.tensor.matmul(gsq_ps[:], goh[:], hsq[:],
                             start=(b == 0), stop=(b == nblk - 1))
            tb += TT

        # ---- stage D: per-graph stats ------------------------------------
        stats = res.tile([G, 2, D], BF16)    # q' | rstd*gamma
        mean = pd.tile([G, D], FP32, tag="mean")
        nc.vector.tensor_scalar(mean[:], gsum_ps[:], invc_sb[:, 0:1], None,
                                op0=OP.mult)
        ex2 = pd.tile([G, D], FP32, tag="ex2")
        nc.vector.tensor_scalar(ex2[:], gsq_ps[:], invc_sb[:, 0:1], None,
                                op0=OP.mult)
        meansc = pd.tile([G, D], FP32, tag="meansc")
        nc.vector.tensor_tensor(meansc[:], mean[:], alphar_sb[:], op=OP.mult)
        t2 = pd.tile([G, D], FP32, tag="t2")
        nc.vector.scalar_tensor_tensor(t2[:], mean[:], 2.0, meansc[:],
                                       op0=OP.mult, op1=OP.subtract)
        var = pd.tile([G, D], FP32, tag="var")
        nc.vector.tensor_tensor(var[:], meansc[:], t2[:], op=OP.mult)
        nc.vector.tensor_tensor(var[:], ex2[:], var[:], op=OP.subtract)
        nc.vector.tensor_scalar(var[:], var[:], EPS, None, op0=OP.add)
        sd = pd.tile([G, D], FP32, tag="sd")
        nc.scalar.activation(sd[:], var[:], ACTF.Sqrt)
        rstd = pd.tile([G, D], FP32, tag="rstd")
        nc.vector.reciprocal(rstd[:], sd[:])
        nc.vector.tensor_tensor(stats[:, 1, :], rstd[:], gammar_sb[:],
                                op=OP.mult)
        # q = meansc * (rstd*gamma) - beta  -> out = h*s - q
        nc.vector.tensor_tensor(stats[:, 0, :], meansc[:],
                                stats[:, 1, :], op=OP.mult)
        nc.vector.tensor_tensor(stats[:, 0, :], stats[:, 0, :],
                                br_sb[:], op=OP.subtract)

        # ---- stage E: normalize + relu + out -----------------------------
        # per-node (q, rstd*gamma) fetched by dma_gather from a G-row DRAM
        # stats table, indexed by the node's graph id
        pagg_cm.__exit__(None, None, None)
        pacc_cm.__exit__(None, None, None)
        dstats = dram.tile([G, 2 * D], BF16)
        nc.sync.dma_start(dstats[:], stats[:].rearrange("g a d -> g (a d)"))
        EB = 5
        for b0 in range(0, nblk, EB):
            eb = min(EB, nblk - b0)
            ixg = pidx.tile([P, 8 * EB], I16, tag="ixg")
            nc.sync.dma_start(ixg[:, :8 * eb],
                              rep[:, L0 + L1 + 8 * b0:L0 + L1 + 8 * (b0 + eb)])
            bcg = ptmp.tile([P, EB, 2 * D], BF16, tag="bcg")
            nc.gpsimd.dma_gather(
                out_ap=bcg[:, :eb, :], in_ap=dstats[:],
                idxs_ap=ixg[:, :8 * eb],
                num_idxs=P * eb, num_idxs_reg=P * eb, elem_size=2 * D,
                single_packet=False)
            for j in range(eb):
                b = b0 + j
                hc = psm.tile([P, D], FP32, tag="hc")
                nc.vector.tensor_tensor(hc[:], hb_all[:, b, :],
                                        bcg[:, j, D:2 * D], op=OP.mult)
                nc.vector.tensor_tensor(hc[:], hc[:], bcg[:, j, 0:D],
                                        op=OP.subtract)
                ho = psm.tile([P, D], BF16, tag="ho")
                nc.scalar.activation(ho[:], hc[:], ACTF.Relu)
                nc.sync.dma_start(hout.ap()[b * P:(b + 1) * P, :], ho[:])

    return nc

# ======================= entry point =======================


def kernel(**inputs) -> np.ndarray:
    inputs = {k: np.asarray(v) for k, v in inputs.items()}
    lay = build(inputs["edge_index"].astype(np.int64),
                inputs["batch"].astype(np.int64))
    meta, in_maps = make_inputs(inputs, lay)

    nc = bacc.Bacc("TRN2", target_bir_lowering=False, debug=False,
                   num_devices=NCORES)
    build_program(nc, meta)
    nc.compile()
    res = bass_utils.run_bass_kernel_spmd(nc, in_maps,
                                          core_ids=list(range(NCORES)))
    outs = [res.results[c]["hout"] for c in range(NCORES)]
    kernel.last = dict(nc=nc, in_maps=in_maps, lay=lay, meta=meta)
    return unshard(lay, outs)


# revision 27
# speedup vs baseline: 5.8597x; 1.0218x over previous
"""EGConv layer (gnn_message_passing) on 8 Trainium2 NeuronCores.

Self-contained: kernel(**inputs) -> np.ndarray [50000, 256] float32.

Strategy: graph-aligned 1D node partition over 8 cores (GraphNorm fully
core-local), per-core degree-sorted node permutation, dst-sorted edge
streams. Each core computes the bases rows of only ITS nodes (plus comb
weights) from its local node shard; a device AllGather assembles the
full [8Q, BF] bf16 bases table on every core. Messages are fetched by
dma_gather from two int16-indexable halves of that table (split at the
core-aligned row 4Q); sum/sym aggregation via block-level one-hot
matmuls on the tensor engine; max via slot-layout gather + halve +
strided max-reduce; per-node einsum in bf16 on the vector engine;
GraphNorm via per-graph one-hot matmuls. Host->device traffic is
minimized: inputs packed into three dtype-blobs (bf16/int16/fp32),
gather-index streams shipped 16-row (replicated to 128 on device),
output returned in bf16. The SPMD program is identical across cores;
all per-core variation is in the data.
"""
import sys
for _p in ("/opt/trn_rl_repo", "/root/.axon_site/_ro/trn_rl_repo"):
    if _p not in sys.path:
        sys.path.insert(0, _p)

import os
import numpy as np
import ml_dtypes
from contextlib import ExitStack

import concourse.bass as bass
import concourse.mybir as mybir
import concourse.tile as tile
from concourse import bacc, bass_utils

BFNP = ml_dtypes.bfloat16

# ======================= host-side graph preprocessing =======================

N, E, D = 50000, 800000, 256
H, B, A = 8, 4, 3
F = D // H          # 32
BF = B * F          # 128
G = 64
EPS = 1e-5
NCORES = 8
P = 128
NEG = -1e30


def build(edge_index: np.ndarray, batch: np.ndarray):
    """edge_index [2,E] int32, batch [N] int32 sorted. Returns layout dict."""
    src_all = np.concatenate([edge_index[0], np.arange(N, dtype=np.int64)])
    dst_all = np.concatenate([edge_index[1], np.arange(N, dtype=np.int64)])

    deg = np.bincount(dst_all, minlength=N).astype(np.float64)
    dinv = np.where(deg > 0, 1.0 / np.sqrt(deg), 0.0).astype(np.float32)
    symw_all = (dinv[src_all] * dinv[dst_all]).astype(np.float32)

    # graph-aligned 8-way shard
    gcnt = np.bincount(batch, minlength=G)
    gend = np.cumsum(gcnt)            # node index where graph g ends
    cuts = [0]
    for c in range(1, NCORES):
        target = round(N * c / NCORES)
        gi = np.argmin(np.abs(gend - target))
        cuts.append(int(gend[gi]))
    cuts.append(N)
    cuts = sorted(set(cuts))
    assert len(cuts) == NCORES + 1, cuts
    cuts_a = np.asarray(cuts, dtype=np.int64)
    src_core = np.searchsorted(cuts_a, src_all, side="right") - 1

    # pass 1: per-core degree-sorted permutation
    cores = []
    for c in range(NCORES):
        n0, n1 = cuts[c], cuts[c + 1]
        nloc = n1 - n0
        local_deg = deg[n0:n1]
        # secondary key: T0-range in-degree (srcs on cores 0-3), to tighten
        # per-range slot rectangles
        ldeg0 = np.bincount(dst_all[(dst_all >= n0) & (dst_all < n1)
                                    & (src_core < 4)] - n0,
                            minlength=nloc).astype(np.float64)
        perm = np.lexsort((-ldeg0, -local_deg)).astype(np.int64)  # desc
        gperm = perm + n0                      # new local id -> global id
        inv = np.empty(nloc, dtype=np.int64)
        inv[perm] = np.arange(nloc)            # orig local -> new local id
        cores.append(dict(n0=n0, n1=n1, nloc=nloc, gperm=gperm, inv=inv))

    maxloc = max(c["nloc"] for c in cores)
    Q = (maxloc // P + 1) * P                  # strictly > every nloc
    nblk = Q // P
    SPL = 4 * Q                                # T0/T1 split row (core-aligned)
    assert SPL - 1 <= 32767 and 4 * Q - 1 <= 32767

    ginv = np.empty(N, dtype=np.int64)         # orig global -> permuted row
    for c, core in enumerate(cores):
        ginv[core["gperm"]] = c * Q + np.arange(core["nloc"])

    # pass 2: per-core edge streams + global Tr/Sr
    nR = 2
    for core in cores:
        n0, n1 = core["n0"], core["n1"]
        emask = (dst_all >= n0) & (dst_all < n1)
        esrc = src_all[emask]
        edstl = core["inv"][dst_all[emask] - n0]   # new local dst id
        esym = symw_all[emask]
        order = np.argsort(edstl, kind="stable")
        core["esrc"], core["edstl"], core["esym"] = \
            esrc[order], edstl[order], esym[order]
        core["erow"] = ginv[core["esrc"]]          # permuted source row

    Tr = np.zeros((nR, nblk), dtype=np.int64)
    Sr = np.zeros((nR, nblk), dtype=np.int64)
    for c in cores:
        blk = c["edstl"] // P
        rng = (c["erow"] >= SPL).astype(np.int64)
        for r in range(nR):
            cnt = np.bincount(blk[rng == r], minlength=nblk)
            Tr[r] = np.maximum(Tr[r], (cnt + P - 1) // P)
            dl = c["edstl"][rng == r]
            nd = np.bincount(dl, minlength=nblk * P).reshape(nblk, P)
            Sr[r] = np.maximum(Sr[r], nd.max(axis=1))
    Tr = np.maximum(Tr, 1)
    Sr = np.maximum(Sr, 1)

    PAD0, PAD1 = SPL - 1, 4 * Q - 1   # NEG tail rows (cores 3 / 7), per-range

    sumTT = int((Tr[0] + Tr[1]).sum())
    for c in cores:
        nloc = c["nloc"]
        dstl_t = np.full((P, sumTT), -1.0, dtype=BFNP)
        symw_t = np.zeros((P, sumTT), dtype=BFNP)
        flat_r = [[]]          # single per-block-interleaved stream
        blk = c["edstl"] // P
        rng = (c["erow"] >= SPL).astype(np.int64)
        tcol = 0
        for b in range(nblk):
            for r in range(nR):
                m = (blk == b) & (rng == r)
                srow = c["erow"][m] - (SPL if r else 0)
                dl = c["edstl"][m] - b * P
                sw = c["esym"][m]
                k = len(srow)
                T, S = int(Tr[r][b]), int(Sr[r][b])
                pad = PAD1 if r else PAD0
                ef = np.full(P * T, pad, dtype=np.int64)
                ef[:k] = srow
                flat_r[0].append(ef)
                cols = tcol + np.arange(k) // P
                rows = np.arange(k) % P
                dstl_t[rows, cols] = dl.astype(np.float32)
                symw_t[rows, cols] = sw
                tcol += T
                sf = np.full(P * S, pad, dtype=np.int64)
                if k:
                    marks = np.flatnonzero(np.diff(dl, prepend=-1))
                    slot = np.arange(k) - np.repeat(marks, np.diff(
                        np.append(marks, k)))
                    sf[slot * P + dl] = srow
                    # pad slots of nodes that HAVE >=1 edge in this range:
                    # duplicate the node's first edge (max unchanged, avoids
                    # a NEG-row fetch)
                    first = np.full(P, -1, dtype=np.int64)
                    first[dl[marks]] = srow[marks]
                    degr = np.zeros(P, dtype=np.int64)
                    dcnt = np.diff(np.append(marks, k))
                    degr[dl[marks]] = dcnt
                    for s in range(S):
                        lane = np.flatnonzero((degr > 0) & (degr <= s))
                        sf[s * P + lane] = first[lane]
                flat_r[0].append(sf)
        fl = np.concatenate(flat_r[0])
        L = len(fl)
        i16_01 = np.zeros((16, L // 16), dtype=np.int16)
        i16_01[np.arange(L) % 16, np.arange(L) // 16] = fl

        gl0 = batch[c["n0"]]
        ngid = np.full(nblk * P, -1.0, dtype=np.float32)
        ngid[:nloc] = (batch[c["gperm"]] - gl0).astype(np.float32)
        gid_t = ngid.reshape(nblk, P).T.copy()

        # stage-E stats-gather index stream (graph id per node, block-major)
        gfl = np.where(ngid < 0, 0, ngid).astype(np.int64)
        L2f = nblk * P
        w2 = np.zeros((16, L2f // 16), dtype=np.int16)
        w2[np.arange(L2f) % 16, np.arange(L2f) // 16] = gfl
        gidx16 = w2

        nmv = np.zeros(nblk * P, dtype=np.float32)
        nmv[nloc:] = NEG
        nmask_t = nmv.reshape(nblk, P).T.copy()

        icnt = np.ones((G, 1), dtype=np.float32)
        glo = np.bincount((batch[c["n0"]:c["n1"]] - gl0), minlength=G)
        icnt[glo > 0, 0] = (1.0 / glo[glo > 0]).astype(np.float32)
        invc = np.ones((G, 1), dtype=np.float32)
        invc[:icnt.shape[0]] = icnt

        c.update(dstl_t=dstl_t, symw_t=symw_t, i16_01=i16_01,
                 gidx16=gidx16, gid_t=gid_t, nmask_t=nmask_t, invcnt=invc)

    return dict(cores=cores, nblk=nblk, Q=Q, Tr=Tr, Sr=Sr, cuts=cuts)


def unshard(layout, per_core_out):
    full = np.zeros((N, D), dtype=np.float32)
    for c, out in zip(layout["cores"], per_core_out):
        full[c["gperm"]] = out[:c["nloc"]].astype(np.float32)
    return full

# ============ input-map construction ============


def to_bf16(x):
    return np.asarray(x, np.float32).astype(BFNP)


def make_inputs(inputs, lay):
    """inputs: dict of full np arrays. lay: build output.
    Returns (meta, in_maps list of 8 dicts)."""
    Q = lay["Q"]
    nblk = lay["nblk"]

    node = np.asarray(inputs["node"], np.float32)
    wb = to_bf16(inputs["W_bases"])                       # [D, BF]
    wc = to_bf16(inputs["W_comb"])                        # [D, HBA]
    bcomb = np.tile(np.asarray(inputs["b_comb"], np.float32)[None, :], (P, 1))
    cbias = np.tile(np.asarray(inputs["conv_bias"], np.float32)[None, :], (P, 1))
    alphar = np.tile(np.asarray(inputs["gn_mean_scale"], np.float32)[None, :], (G, 1))
    gammar = np.tile(np.asarray(inputs["gn_weight"], np.float32)[None, :], (G, 1))
    br = np.tile(np.asarray(inputs["gn_bias"], np.float32)[None, :], (G, 1))

    meta = dict(Q=Q, nblk=nblk,
                Tr0=[int(x) for x in lay["Tr"][0]],
                Tr1=[int(x) for x in lay["Tr"][1]],
                Sr0=[int(x) for x in lay["Sr"][0]],
                Sr1=[int(x) for x in lay["Sr"][1]])

    in_maps = []
    for c in lay["cores"]:
        ntl = np.zeros((D, Q), BFNP)
        ntl[:, :c["nloc"]] = to_bf16(node[c["gperm"]]).T
        blob_bf = np.concatenate([
            ntl.ravel(), wb.ravel(), wc.ravel(),
            c["dstl_t"].ravel(), c["symw_t"].ravel()])
        blob_i16 = np.hstack([c["i16_01"], c["gidx16"]]).ravel()
        blob_f32 = np.concatenate([
            bcomb.ravel(), c["gid_t"].ravel(), c["invcnt"].ravel(),
            alphar.ravel(), gammar.ravel(), br.ravel(), cbias.ravel(),
            c["nmask_t"].ravel()])
        in_maps.append(dict(blob_bf=blob_bf, blob_i16=blob_i16,
                            blob_f32=blob_f32))
    return meta, in_maps

# ============ device program ============

FP32 = mybir.dt.float32
BF16 = mybir.dt.bfloat16
I32 = mybir.dt.int32
I16 = mybir.dt.int16
AX = mybir.AxisListType
OP = mybir.AluOpType
ACTF = mybir.ActivationFunctionType
HBA = H * B * A   # 96
K = B * A         # 12


def build_program(nc, meta):
    Q = meta["Q"]
    nblk = meta["nblk"]
    Tr0, Tr1 = list(meta["Tr0"]), list(meta["Tr1"])
    Sr0, Sr1 = list(meta["Sr0"]), list(meta["Sr1"])
    sumT = sum(Tr0) + sum(Tr1)
    L0 = sum(8 * (t + s) for t, s in zip(Tr0, Sr0))   # cols of [16, L0]
    L1 = sum(8 * (t + s) for t, s in zip(Tr1, Sr1))
    L2 = 8 * nblk                                      # stage-E gid stream
    TTmax = max(t0 + t1 for t0, t1 in zip(Tr0, Tr1))
    SPL = 4 * Q

    # ---- external blobs ---------------------------------------------------
    LBF = D * Q + D * BF + D * HBA + 2 * P * sumT
    LF32 = P * HBA + P * nblk + G + 3 * G * D + P * D + P * nblk
    blob_bf = nc.dram_tensor("blob_bf", [LBF], BF16, kind="ExternalInput")
    blob_i16 = nc.dram_tensor("blob_i16", [16 * (L0 + L1 + L2)], I16,
                              kind="ExternalInput")
    blob_f32 = nc.dram_tensor("blob_f32", [LF32], FP32, kind="ExternalInput")
    hout = nc.dram_tensor("hout", [Q, D], BF16, kind="ExternalOutput")

    o_ntl = 0
    o_wb = o_ntl + D * Q
    o_wc = o_wb + D * BF
    o_dstl = o_wc + D * HBA
    o_symw = o_dstl + P * sumT
    f_bcomb = 0
    f_gid = f_bcomb + P * HBA
    f_invc = f_gid + P * nblk
    f_alphar = f_invc + G
    f_gammar = f_alphar + G * D
    f_br = f_gammar + G * D
    f_cbias = f_br + G * D
    f_nmask = f_cbias + P * D

    with ExitStack() as ctx:
        tc = ctx.enter_context(tile.TileContext(nc))
        dram = ctx.enter_context(tc.tile_pool(name="dram", bufs=1, space="DRAM"))
        res = ctx.enter_context(tc.tile_pool(name="res", bufs=1))
        pa = ctx.enter_context(tc.tile_pool(name="pa", bufs=3))
        pgath = ctx.enter_context(tc.tile_pool(name="pgath", bufs=2))
        pidx = ctx.enter_context(tc.tile_pool(name="pidx", bufs=2))
        poh = ctx.enter_context(tc.tile_pool(name="poh", bufs=2))
        ptmp = ctx.enter_context(tc.tile_pool(name="ptmp", bufs=2))
        psm = ctx.enter_context(tc.tile_pool(name="psm", bufs=2))
        pd = ctx.enter_context(tc.tile_pool(name="pd", bufs=1))

        # ---- constants / resident tiles ----------------------------------
        wb_sb = res.tile([P, 2, BF], BF16)
        nc.sync.dma_start(wb_sb[:], blob_bf.ap()[o_wb:o_wb + D * BF]
                          .rearrange("(a p f) -> p a f", p=P, f=BF))
        wc_sb = res.tile([P, 2, HBA], BF16)
        nc.sync.dma_start(wc_sb[:], blob_bf.ap()[o_wc:o_wc + D * HBA]
                          .rearrange("(a p f) -> p a f", p=P, f=HBA))
        dstl_sb = res.tile([P, sumT], BF16)
        nc.sync.dma_start(dstl_sb[:], blob_bf.ap()[o_dstl:o_dstl + P * sumT]
                          .rearrange("(p t) -> p t", p=P))
        symw_bf = res.tile([P, sumT], BF16)
        nc.sync.dma_start(symw_bf[:], blob_bf.ap()[o_symw:o_symw + P * sumT]
                          .rearrange("(p t) -> p t", p=P))
        symw_sb = res.tile([P, sumT], FP32)   # scalar-engine scale must be FP32
        nc.vector.tensor_copy(symw_sb[:], symw_bf[:])
        bcomb_sb = res.tile([P, HBA], FP32)
        nc.sync.dma_start(bcomb_sb[:], blob_f32.ap()[f_bcomb:f_bcomb + P * HBA]
                          .rearrange("(p t) -> p t", p=P))
        gid_sb = res.tile([P, nblk], FP32)
        nc.sync.dma_start(gid_sb[:], blob_f32.ap()[f_gid:f_gid + P * nblk]
                          .rearrange("(p t) -> p t", p=P))
        invc_sb = res.tile([G, 1], FP32)
        nc.sync.dma_start(invc_sb[:], blob_f32.ap()[f_invc:f_invc + G]
                          .rearrange("(p t) -> p t", p=G))
        alphar_sb = res.tile([G, D], FP32)
        nc.sync.dma_start(alphar_sb[:], blob_f32.ap()[f_alphar:f_alphar + G * D]
                          .rearrange("(p t) -> p t", p=G))
        gammar_sb = res.tile([G, D], FP32)
        nc.sync.dma_start(gammar_sb[:], blob_f32.ap()[f_gammar:f_gammar + G * D]
                          .rearrange("(p t) -> p t", p=G))
        br_sb = res.tile([G, D], FP32)
        nc.sync.dma_start(br_sb[:], blob_f32.ap()[f_br:f_br + G * D]
                          .rearrange("(p t) -> p t", p=G))
        cbias_sb = res.tile([P, D], FP32)
        nc.sync.dma_start(cbias_sb[:], blob_f32.ap()[f_cbias:f_cbias + P * D]
                          .rearrange("(p t) -> p t", p=P))
        nmask_sb = res.tile([P, nblk], FP32)
        nc.sync.dma_start(nmask_sb[:], blob_f32.ap()[f_nmask:f_nmask + P * nblk]
                          .rearrange("(p t) -> p t", p=P))

        iota_i = res.tile([P, P], I32)
        nc.gpsimd.iota(iota_i[:], pattern=[[1, P]], base=0, channel_multiplier=0)
        iota_f = res.tile([P, P], FP32)
        nc.vector.tensor_copy(iota_f[:], iota_i[:])
        iota_bf = res.tile([P, P], BF16)
        nc.vector.tensor_copy(iota_bf[:], iota_i[:])
        iota_exp = res.tile([P, P, TTmax], BF16)
        nc.scalar.copy(iota_exp[:],
                       iota_bf[:].unsqueeze(2).broadcast_to([P, P, TTmax]))

        comb_sb = res.tile([P, nblk, HBA], BF16)
        hb_all = res.tile([P, nblk, D], BF16)

        # ---- 16->128 replication of gather-index streams ------------------
        # layout: [0,L0) range0, [L0,L0+L1) range1, [L0+L1,..) stage-E gids
        rep = dram.tile([P, L0 + L1 + L2], I16)
        for kk in range(8):
            nc.sync.dma_start(rep[16 * kk:16 * (kk + 1), :],
                              blob_i16.ap().rearrange("(a l) -> a l", a=16))

        # ---- stage A: local bases segment + comb, then AllGather ----------
        mybases = dram.tile([Q, BF], BF16)
        bases_all = dram.tile([NCORES * Q, BF], BF16)
        pab_cm = tc.tile_pool(name="pab", bufs=4, space="PSUM")
        pab = pab_cm.__enter__()
        pcb_cm = tc.tile_pool(name="pcb", bufs=2, space="PSUM")
        pcb = pcb_cm.__enter__()

        ntl_ap = blob_bf.ap()[o_ntl:o_ntl + D * Q].rearrange(
            "(a p n) -> p a n", p=P, n=Q)
        for b0 in range(0, nblk, 2):
            bn = min(2, nblk - b0)
            lt2 = pa.tile([P, 2, 2 * P], BF16, tag="lt")
            nc.sync.dma_start(lt2[:, :, :bn * P],
                              ntl_ap[:, :, b0 * P:(b0 + bn) * P])
            for j in range(bn):
                b = b0 + j
                ps = pab.tile([P, BF], FP32, tag="ab")
                nc.tensor.matmul(ps[:], lt2[:, 0, j * P:(j + 1) * P],
                                 wb_sb[:, 0, :], start=True, stop=False)
                nc.tensor.matmul(ps[:], lt2[:, 1, j * P:(j + 1) * P],
                                 wb_sb[:, 1, :], start=False, stop=True)
                ob = pa.tile([P, BF], BF16, tag="ob")
                nc.vector.tensor_scalar(ob[:], ps[:], nmask_sb[:, b:b + 1],
                                        None, op0=OP.add)
                nc.sync.dma_start(mybases[b * P:(b + 1) * P, :], ob[:])
                cps = pcb.tile([P, HBA], FP32, tag="cps")
                nc.tensor.matmul(cps[:], lt2[:, 0, j * P:(j + 1) * P],
                                 wc_sb[:, 0, :], start=True, stop=False)
                nc.tensor.matmul(cps[:], lt2[:, 1, j * P:(j + 1) * P],
                                 wc_sb[:, 1, :], start=False, stop=True)
                nc.vector.tensor_tensor(comb_sb[:, b, :], cps[:],
                                        bcomb_sb[:], op=OP.add)

        pcb_cm.__exit__(None, None, None)
        pab_cm.__exit__(None, None, None)

        nc.gpsimd.collective_compute(
            "AllGather", OP.bypass,
            replica_groups=[list(range(NCORES))],
            ins=[mybases[:].opt()],
            outs=[bases_all[:].opt()])

        # ---- stage C: gather + aggregate + einsum + stats -----------------
        pacc_cm = tc.tile_pool(name="pacc", bufs=1, space="PSUM")
        pacc = pacc_cm.__enter__()
        pagg_cm = tc.tile_pool(name="pagg", bufs=2, space="PSUM")
        pagg = pagg_cm.__enter__()
        gsum_ps = pacc.tile([G, D], FP32)
        gsq_ps = pacc.tile([G, D], FP32)

        CH = 64                       # <=8192 idxs per dma_gather call
        c0 = 0
        tb = 0
        for b in range(nblk):
            T0, T1 = Tr0[b], Tr1[b]
            S0, S1 = Sr0[b], Sr1[b]
            W0, W1 = T0 + S0, T1 + S1
            TT = T0 + T1
            # gw[:, 1, :, :] = gathered messages; gw[:, 0, tile cols, :] =
            # symw-weighted messages (slot cols of plane 0 unused)
            gw = pgath.tile([P, 2, W0 + W1, BF], BF16, tag="gath")
            if b < 2:
                nc.gpsimd.memset(gw[:], 0.0)
            ix = pidx.tile([P, 8 * (W0 + W1)], I16, tag="ix")
            nc.sync.dma_start(ix[:], rep[:, c0:c0 + 8 * (W0 + W1)])
            for w0 in range(0, W0, CH):
                w = min(CH, W0 - w0)
                nc.gpsimd.dma_gather(
                    out_ap=gw[:, 1, w0:w0 + w, :], in_ap=bases_all[0:SPL, :],
                    idxs_ap=ix[:, 8 * w0:8 * (w0 + w)],
                    num_idxs=P * w, num_idxs_reg=P * w, elem_size=BF,
                    single_packet=False)
            for w1 in range(0, W1, CH):
                w = min(CH, W1 - w1)
                nc.gpsimd.dma_gather(
                    out_ap=gw[:, 1, W0 + w1:W0 + w1 + w, :],
                    in_ap=bases_all[SPL:NCORES * Q, :],
                    idxs_ap=ix[:, 8 * (W0 + w1):8 * (W0 + w1 + w)],
                    num_idxs=P * w, num_idxs_reg=P * w, elem_size=BF,
                    single_packet=False)
            c0 += 8 * (W0 + W1)

            # weighted copies of the tile columns (slot cols skipped)
            nc.vector.tensor_tensor(
                gw[:, 0, 0:T0, :], gw[:, 1, 0:T0, :],
                symw_sb[:, tb:tb + T0].unsqueeze(2).broadcast_to([P, T0, BF]),
                op=OP.mult)
            nc.vector.tensor_tensor(
                gw[:, 0, W0:W0 + T1, :], gw[:, 1, W0:W0 + T1, :],
                symw_sb[:, tb + T0:tb + TT].unsqueeze(2)
                .broadcast_to([P, T1, BF]),
                op=OP.mult)

            # block-level one-hot builds: oh[p_edge, x, t]
            oh = poh.tile([P, P, TTmax], BF16, tag="oh")
            nc.vector.tensor_tensor(
                oh[:, :, :TT],
                dstl_sb[:, tb:tb + TT].unsqueeze(1).broadcast_to([P, P, TT]),
                iota_exp[:, :, :TT], op=OP.is_equal)

            # one matmul per edge tile: moving [P, 2, BF] = (msg | w*msg)
            ps_c = pagg.tile([P, 2, BF], FP32, tag="aggc")
            for t in range(TT):
                mcol = t if t < T0 else W0 + (t - T0)
                nc.tensor.matmul(ps_c[:], oh[:, :, t], gw[:, :, mcol, :],
                                 start=(t == 0), stop=(t == TT - 1))
            # max: halve (overlap-safe) then one strided reduce over both
            # ranges into aggT[:, :, 8:12]
            m0, m1 = (S0 + 1) // 2, (S1 + 1) // 2
            hmax = ptmp.tile([P, m0 + m1, BF], BF16, tag="hmax")
            nc.vector.tensor_tensor(hmax[:, :m0, :],
                                    gw[:, 1, T0:T0 + m0, :],
                                    gw[:, 1, T0 + S0 - m0:T0 + S0, :],
                                    op=OP.max)
            nc.vector.tensor_tensor(hmax[:, m0:m0 + m1, :],
                                    gw[:, 1, W0 + T1:W0 + T1 + m1, :],
                                    gw[:, 1, W0 + W1 - m1:W0 + W1, :],
                                    op=OP.max)
            aggT = psm.tile([P, F, K], BF16, tag="aggT")
            nc.vector.tensor_reduce(
                aggT[:, :, 2 * B:3 * B].transpose([0, 2, 1]),
                hmax[:].rearrange("p s (bb f) -> p (bb f) s", bb=B),
                axis=AX.X, op=OP.max, opt_input=False)
            # sym (a=0) / sum (a=1) from psum, transposed to [P, F, b]
            nc.scalar.copy(
                aggT[:, :, 0:2 * B].rearrange("p f (a bb) -> p a bb f", a=2),
                ps_c[:].rearrange("p a (bb f) -> p a bb f", bb=B))

            # einsum: tmp[p,h,f,k] = aggT[p,f,k] * comb[p,h,k]; reduce k
            tmp = ptmp.tile([P, H, F, K], BF16, tag="tmp")
            nc.vector.tensor_tensor(
                tmp[:],
                aggT[:].unsqueeze(1).broadcast_to([P, H, F, K]),
                comb_sb[:, b, :].rearrange("p (h k) -> p h k", h=H)
                .unsqueeze(2).broadcast_to([P, H, F, K]),
                op=OP.mult)
            hbt = psm.tile([P, D], FP32, tag="hbt")
            nc.vector.tensor_reduce(hbt[:], tmp[:], axis=AX.X, op=OP.add,
                                    opt_input=False)
            nc.vector.tensor_tensor(hb_all[:, b, :], hbt[:], cbias_sb[:],
                                    op=OP.add)
            hsq = psm.tile([P, D], BF16, tag="hsq")
            nc.scalar.square(hsq[:], hb_all[:, b, :])

            # graph one-hot + stats
            goh = psm.tile([P, G], BF16, tag="goh")
            nc.vector.tensor_scalar(goh[:], iota_f[:, :G],
                                    gid_sb[:, b:b + 1], None, op0=OP.is_equal)
            nc.tensor.matmul(gsum_ps[:], goh[:], hb_all[:, b, :],
                             start=(b == 0), stop=(b == nblk - 1))
            nc.tensor.matmul(gsq_ps[:], goh[:], hsq[:],
                             start=(b == 0), stop=(b == nblk - 1))
            tb += TT

        # ---- stage D: per-graph stats ------------------------------------
        stats = res.tile([G, 2, D], BF16)    # q' | rstd*gamma
        mean = pd.tile([G, D], FP32, tag="mean")
        nc.vector.tensor_scalar(mean[:], gsum_ps[:], invc_sb[:, 0:1], None,
                                op0=OP.mult)
        ex2 = pd.tile([G, D], FP32, tag="ex2")
        nc.vector.tensor_scalar(ex2[:], gsq_ps[:], invc_sb[:, 0:1], None,
                                op0=OP.mult)
        meansc = pd.tile([G, D], FP32, tag="meansc")
        nc.vector.tensor_tensor(meansc[:], mean[:], alphar_sb[:], op=OP.mult)
        t2 = pd.tile([G, D], FP32, tag="t2")
        nc.vector.scalar_tensor_tensor(t2[:], mean[:], 2.0, meansc[:],
                                       op0=OP.mult, op1=OP.subtract)
        var = pd.tile([G, D], FP32, tag="var")
        nc.vector.tensor_tensor(var[:], meansc[:], t2[:], op=OP.mult)
        nc.vector.tensor_tensor(var[:], ex2[:], var[:], op=OP.subtract)
        nc.vector.tensor_scalar(var[:], var[:], EPS, None, op0=OP.add)
        sd = pd.tile([G, D], FP32, tag="sd")
        nc.scalar.activation(sd[:], var[:], ACTF.Sqrt)
        rstd = pd.tile([G, D], FP32, tag="rstd")
        nc.vector.reciprocal(rstd[:], sd[:])
        nc.vector.tensor_tensor(stats[:, 1, :], rstd[:], gammar_sb[:],
                                op=OP.mult)
        # q = meansc * (rstd*gamma) - beta  -> out = h*s - q
        nc.vector.tensor_tensor(stats[:, 0, :], meansc[:],
                                stats[:, 1, :], op=OP.mult)
        nc.vector.tensor_tensor(stats[:, 0, :], stats[:, 0, :],
                                br_sb[:], op=OP.subtract)

        # ---- stage E: normalize + relu + out -----------------------------
        # per-node (q, rstd*gamma) fetched by dma_gather from a G-row DRAM
        # stats table, indexed by the node's graph id
        pagg_cm.__exit__(None, None, None)
        pacc_cm.__exit__(None, None, None)
        dstats = dram.tile([G, 2 * D], BF16)
        nc.sync.dma_start(dstats[:], stats[:].rearrange("g a d -> g (a d)"))
        EB = 5
        for b0 in range(0, nblk, EB):
            eb = min(EB, nblk - b0)
            ixg = pidx.tile([P, 8 * EB], I16, tag="ixg")
            nc.sync.dma_start(ixg[:, :8 * eb],
                              rep[:, L0 + L1 + 8 * b0:L0 + L1 + 8 * (b0 + eb)])
            bcg = ptmp.tile([P, EB, 2 * D], BF16, tag="bcg")
            nc.gpsimd.dma_gather(
                out_ap=bcg[:, :eb, :], in_ap=dstats[:],
                idxs_ap=ixg[:, :8 * eb],
                num_idxs=P * eb, num_idxs_reg=P * eb, elem_size=2 * D,
                single_packet=False)
            for j in range(eb):
                b = b0 + j
                hc = psm.tile([P, D], FP32, tag="hc")
                nc.vector.tensor_tensor(hc[:], hb_all[:, b, :],
                                        bcg[:, j, D:2 * D], op=OP.mult)
                nc.vector.tensor_tensor(hc[:], hc[:], bcg[:, j, 0:D],
                                        op=OP.subtract)
                ho = psm.tile([P, D], BF16, tag="ho")
                nc.scalar.activation(ho[:], hc[:], ACTF.Relu)
                nc.sync.dma_start(hout.ap()[b * P:(b + 1) * P, :], ho[:])

    return nc

# ======================= entry point =======================


def kernel(**inputs) -> np.ndarray:
    inputs = {k: np.asarray(v) for k, v in inputs.items()}
    lay = build(inputs["edge_index"].astype(np.int64),
                inputs["batch"].astype(np.int64))
    meta, in_maps = make_inputs(inputs, lay)

    nc = bacc.Bacc("TRN2", target_bir_lowering=False, debug=False,
                   num_devices=NCORES)
    build_program(nc, meta)
    nc.compile()
    res = bass_utils.run_bass_kernel_spmd(nc, in_maps,
                                          core_ids=list(range(NCORES)))
    outs = [res.results[c]["hout"] for c in range(NCORES)]
    kernel.last = dict(nc=nc, in_maps=in_maps, lay=lay, meta=meta)
    return unshard(lay, outs)
